# revision 1
# baseline (speedup 1.0000x reference)
"""DCRNN Trainium2 kernel: 8-way node sharding with on-device A^2 operators.

Decomposition (validated in mirror.py):
- A row-normalized on host; per-core operator column-slices R_A = A^T[:, sh],
  R_AT = A[:, sh] fed as inputs. On device, R_A2 = (A^T)^2[:, sh] is computed
  by streaming A once; R_AT2 = A^2[:, sh] is derived from R_A2 via an
  AllToAll block exchange + PE transposes.
- Activations live feature-major per shard: state tiles [H=64, (b, n)=512].
- Each diffused tensor gets a "bundle" [64, (b, op5, n256)] = identity + the
  4 operator applications, produced by f32r matmuls whose lhsT is the
  AllGathered node-major activation [2048, cols] streamed in 128-row chunks
  against the resident operator pair tiles (rhs [128, 512], full f32r rate).
- Projections contract (op, feat) with K=64 W slices against bundle slices;
  gates/cand ACT and the GRU update are row-local DVE/ACT work.
- 10 AllGathers total; xp diffusions are batched up-front (xp is global).

Hardware constraints honored (probed on trn2):
- every instruction <= 1 sync wait -> must build on bacc.Bacc + nc.compile()
  (generate_event_semaphores legalizes)
- f32r matmul inputs must be produced f32r (DMA-bitcast or DVE-copy out)
- 2-input DVE ops and matmul lhsT/rhs need equal base partitions
- DMA cannot read PSUM; transposes bounce PSUM -> DVE copy -> SBUF
"""
import numpy as np
import concourse.bass as bass
import concourse.bacc as bacc
import concourse.tile as tile
from concourse import mybir
from concourse.bass_utils import run_bass_kernel_spmd

F32 = mybir.dt.float32
F32R = mybir.dt.float32r
BF16 = mybir.dt.bfloat16
AF = mybir.ActivationFunctionType

N, H, B, SEQ, L = 2048, 64, 2, 3, 2
W = 8            # cores
NS = N // W      # 256 nodes per shard
KT = N // 128    # 16 contraction tiles
BN = B * NS      # 512 = (b, n) free size
RG = [list(range(W))]
PHASES = [("enc", 0), ("enc", 1), ("dec", 0), ("dec", 1)]  # dram row order
HB_BUFS, RHB_BUFS = 4, 1


def build_program():
    nc = bacc.Bacc(None, num_devices=W, name="dcrnn")

    # ---- DRAM inputs (per core) ----
    r_a = nc.dram_tensor("r_a", [N, NS], F32, kind="ExternalInput")
    r_at = nc.dram_tensor("r_at", [N, NS], F32, kind="ExternalInput")
    a_full = nc.dram_tensor("a_full", [N, N], F32, kind="ExternalInput")
    xp_nm = nc.dram_tensor("xp_nm", [N, SEQ * 128], F32, kind="ExternalInput")
    xp_fm = nc.dram_tensor("xp_fm", [H, SEQ * BN], F32, kind="ExternalInput")
    wg_in = nc.dram_tensor("wg_in", [4, 5 * 128, 2 * H], F32, kind="ExternalInput")
    wc_in = nc.dram_tensor("wc_in", [4, 5 * 128, H], F32, kind="ExternalInput")
    bg_in = nc.dram_tensor("bg_in", [4 * 2 * H, 1], F32, kind="ExternalInput")
    bc_in = nc.dram_tensor("bc_in", [4 * H, 1], F32, kind="ExternalInput")
    wout_in = nc.dram_tensor("wout_in", [H, 1], F32, kind="ExternalInput")
    bout_in = nc.dram_tensor("bout_in", [1, 1], F32, kind="ExternalInput")
    ident_in = nc.dram_tensor("ident_in", [128, 128], F32, kind="ExternalInput")
    out_t = nc.dram_tensor("out", [1, BN], F32, kind="ExternalOutput")

    with tile.TileContext(nc) as tc:
        with (
            tc.tile_pool(name="persist", bufs=1) as persist,
            tc.tile_pool(name="acolp", bufs=2) as acolp,
            tc.tile_pool(name="lhstp", bufs=2) as lhstp,
            tc.tile_pool(name="hbp", bufs=HB_BUFS) as hbp,
            tc.tile_pool(name="rhbp", bufs=RHB_BUFS) as rhbp,
            tc.tile_pool(name="statep", bufs=2) as statep,
            tc.tile_pool(name="hstp", bufs=5) as hstp,
            tc.tile_pool(name="smallp", bufs=2) as smallp,
            tc.tile_pool(name="pdiff", bufs=4, space="PSUM") as pdiff,
            tc.tile_pool(name="pproj", bufs=2, space="PSUM") as pproj,
            tc.tile_pool(name="ptr", bufs=1, space="PSUM") as ptr,
            tc.tile_pool(name="dml", bufs=3, space="DRAM") as dml,
            tc.tile_pool(name="dms", bufs=2, space="DRAM") as dms,
        ):
            uid = [0]

            def nm(pfx):
                uid[0] += 1
                return f"{pfx}{uid[0]}"

            dma_engines = [nc.sync, nc.scalar, nc.gpsimd]

            def dma_eng(i):
                return dma_engines[i % 3]

            # =================================================================
            # helpers
            # =================================================================
            bundles = {}   # name -> (tile, alloc_idx, tag)
            alloc_count = {"hb": 0, "rhb": 0}
            state = {}     # name -> state tile [64, BN]

            def bundle_alloc(name, pool, tag):
                t = pool.tile([H, B, 5, NS], F32R, name=nm("bun_" + name), tag=tag)
                alloc_count[tag] += 1
                bundles[name] = (t, alloc_count[tag], tag)
                return t

            def bundle_get(name):
                t, idx, tag = bundles[name]
                bufs = {"hb": HB_BUFS, "rhb": RHB_BUFS}[tag]
                assert idx > alloc_count[tag] - bufs, \
                    f"bundle {name} slot recycled ({idx} vs {alloc_count[tag]})"
                return t

            def emit_diffusion(src_dram, names, pool_tags):
                """src_dram: node-major [N, 128*len(names)] DRAM AP."""
                Cm = len(names)
                buns, ps = [], []
                for ti, name in enumerate(names):
                    if name in bundles:
                        buns.append(bundle_get(name))
                    else:
                        pool, tag = pool_tags[ti]
                        buns.append(bundle_alloc(name, pool, tag))
                    p1 = pdiff.tile([128, 512], F32, name=nm("p1"), tag="pdiff")
                    p2 = pdiff.tile([128, 512], F32, name=nm("p2"), tag="pdiff")
                    ps.append((p1, p2))
                KC = 4  # kt tiles per readback chunk
                for ck in range(KT // KC):
                    lt = lhstp.tile([128, KC, Cm * 128], F32R, name=nm("lt"), tag="lt")
                    dma_eng(ck).dma_start(
                        lt,
                        src_dram[ck * KC * 128:(ck + 1) * KC * 128, :]
                        .bitcast(F32R).rearrange("(k p) c -> p k c", p=128),
                    )
                    for k2 in range(KC):
                        kt = ck * KC + k2
                        for ti in range(Cm):
                            p1, p2 = ps[ti]
                            lts = lt[:, k2, ti * 128:(ti + 1) * 128]
                            nc.tensor.matmul(p1, lts, rp1[:, kt, :],
                                             start=(kt == 0), stop=(kt == KT - 1))
                            nc.tensor.matmul(p2, lts, rp2[:, kt, :],
                                             start=(kt == 0), stop=(kt == KT - 1))
                for ti in range(Cm):
                    p1, p2 = ps[ti]
                    bun = buns[ti]
                    for b in range(B):
                        # ops (A, A2) -> [:, b, 1:3, :]; (AT, AT2) -> [:, b, 3:5, :]
                        nc.vector.tensor_copy(bun[:, b, 1:3, :], p1[b * H:(b + 1) * H, :])
                        nc.vector.tensor_copy(bun[:, b, 3:5, :], p2[b * H:(b + 1) * H, :])
                return buns

            def set_identity_slot(bun, src_state):
                nc.vector.tensor_copy(
                    bun[:, :, 0, :],
                    src_state.bitcast(F32R).rearrange("p (b n) -> p b n", b=B),
                )

            def emit_allgather(tensors):
                """tensors: state tiles [64, BN] feature-major. Returns gathered
                node-major DRAM tile [N, 128*len(tensors)]."""
                Cg = 128 * len(tensors)
                stg = statep.tile([128, 2, Cg], F32, name=nm("stg"), tag="stg")
                for ti, t in enumerate(tensors):
                    for b in range(B):
                        for nh in range(2):
                            pt = ptr.tile([128, H], F32, name=nm("agt"), tag="ptr")
                            nc.tensor.transpose(
                                pt,
                                t[:, b * NS + nh * 128: b * NS + (nh + 1) * 128],
                                ident[0:H, 0:H],
                            )
                            nc.vector.tensor_copy(
                                stg[:, nh, ti * 128 + b * H: ti * 128 + (b + 1) * H],
                                pt,
                            )
                ag_in = dml.tile([NS, Cg], F32, name=nm("ag_in"), tag="agin")
                nc.sync.dma_start(ag_in.rearrange("(nh p) c -> p nh c", p=128), stg)
                ag_out = dms.tile([N, Cg], F32, name=nm("ag_out"), tag="agout",
                                  addr_space="Shared")
                nc.gpsimd.collective_compute(
                    "AllGather", mybir.AluOpType.bypass, replica_groups=RG,
                    ins=[ag_in.opt()], outs=[ag_out.opt()],
                )
                return ag_out

            def proj_mms(wt, bname):
                bun = bundle_get(bname)
                return [(wt[:, op, :], bun[:, :, op, :]) for op in range(5)]

            def emit_mm_chain(psum_out, mms, start, stop):
                pv = psum_out.rearrange("p (b n) -> p b n", b=B)
                for i, (wap, rap) in enumerate(mms):
                    nc.tensor.matmul(pv, wap, rap,
                                     start=(start and i == 0),
                                     stop=(stop and i == len(mms) - 1))

            def emit_projection(psum_out, wx, wh, parts, out_dim):
                mms = []
                for (bname, pos) in parts:
                    if bname is None:
                        continue
                    mms += proj_mms(wx if pos == "x" else wh, bname)
                assert mms
                emit_mm_chain(psum_out, mms, True, True)

            def emit_cell(ph, l, x_name, h_name, cid):
                key = (ph, l)
                hs_t = state[h_name] if h_name is not None else None
                gps = pproj.tile([2 * H, BN], F32, name=nm("gps"), tag="pproj")
                emit_projection(gps, wgx[key], wgh[key],
                                [(x_name, "x"), (h_name, "h")], 2 * H)
                r_t = statep.tile([H, BN], F32, name=nm("r"), tag="r")
                nc.scalar.activation(r_t, gps[0:H, :], AF.Sigmoid, bias=bgr_sb[key])
                u_t = statep.tile([H, BN], F32, name=nm("u"), tag="u")
                nc.scalar.activation(u_t, gps[H:2 * H, :], AF.Sigmoid, bias=bgu_sb[key])

                cps = pproj.tile([H, BN], F32, name=nm("cps"), tag="pproj")
                if h_name is not None:
                    rh_t = statep.tile([H, BN], F32, name=nm("rh"), tag="rh")
                    nc.vector.tensor_mul(rh_t, r_t, hs_t)
                    rh_name = f"rh_{cid}"
                    # cand x-part first: these matmuls depend only on cached
                    # bundles, so the PE runs them during the rh AllGather
                    if x_name is not None:
                        emit_mm_chain(cps, proj_mms(wcx[key], x_name), True, False)
                    ag = emit_allgather([rh_t])
                    buns = emit_diffusion(ag[:, :], [rh_name], [(rhbp, "rhb")])
                    set_identity_slot(buns[0], rh_t)
                    emit_mm_chain(cps, proj_mms(wch[key], rh_name),
                                  x_name is None, True)
                else:
                    rh_name = None
                    emit_projection(cps, wcx[key], wch[key],
                                    [(x_name, "x")], H)
                cand_t = statep.tile([H, BN], F32, name=nm("cand"), tag="cand")
                nc.scalar.activation(cand_t, cps, AF.Tanh, bias=bc_sb[key])

                hn = hstp.tile([H, BN], F32, name=nm("h"), tag="hst")
                tmp = statep.tile([H, BN], F32, name=nm("tmp"), tag="tmp")
                if h_name is None:
                    nc.vector.tensor_mul(tmp, u_t, cand_t)
                    nc.vector.tensor_sub(hn, cand_t, tmp)        # (1-u)*c
                else:
                    tmp2 = statep.tile([H, BN], F32, name=nm("tmp2"), tag="tmp2")
                    nc.vector.tensor_sub(tmp, hs_t, cand_t)
                    nc.vector.tensor_mul(tmp2, u_t, tmp)
                    nc.vector.tensor_add(hn, cand_t, tmp2)       # c + u*(h-c)
                sname = f"h{l}_{cid}"
                state[sname] = hn
                return sname

            def gather_and_diffuse(names):
                ag = emit_allgather([state[nm_] for nm_ in names])
                buns = emit_diffusion(ag[:, :], names, [(hbp, "hb")] * len(names))
                for bun, nm_ in zip(buns, names):
                    set_identity_slot(bun, state[nm_])

            xp_ps = {}

            def xp_hop(c0, c1, names, hop):
                """hop 0: ops (A, AT) into psum halves [:, 0:NS]; emitted before
                the A2A readback so the PE overlaps the collective.
                hop 1: ops (A2, AT2) into [:, NS:512] + bundle copies."""
                Cm = len(names)
                if hop == 0:
                    xp_ps[c0] = [
                        (pdiff.tile([128, 512], F32, name=nm("p1"), tag="pdiff"),
                         pdiff.tile([128, 512], F32, name=nm("p2"), tag="pdiff"))
                        for _ in range(Cm)]
                ps = xp_ps[c0]
                KC = 4
                for ck in range(KT // KC):
                    lt = lhstp.tile([128, KC, Cm * 128], F32R, name=nm("lt"), tag="lt")
                    dma_eng(ck + hop).dma_start(
                        lt,
                        xp_nm.ap()[ck * KC * 128:(ck + 1) * KC * 128, c0:c1]
                        .bitcast(F32R).rearrange("(k p) c -> p k c", p=128),
                    )
                    for k2 in range(KC):
                        kt = ck * KC + k2
                        for ti in range(Cm):
                            p1, p2 = ps[ti]
                            lts = lt[:, k2, ti * 128:(ti + 1) * 128]
                            col = slice(hop * NS, hop * NS + NS)
                            nc.tensor.matmul(p1[:, col], lts, rp1[:, kt, col],
                                             start=(kt == 0), stop=(kt == KT - 1))
                            nc.tensor.matmul(p2[:, col], lts, rp2[:, kt, col],
                                             start=(kt == 0), stop=(kt == KT - 1))
                if hop == 1:
                    for ti, name in enumerate(names):
                        p1, p2 = ps[ti]
                        bun = bundle_get(name)
                        for b in range(B):
                            nc.vector.tensor_copy(bun[:, b, 1:3, :],
                                                  p1[b * H:(b + 1) * H, :])
                            nc.vector.tensor_copy(bun[:, b, 3:5, :],
                                                  p2[b * H:(b + 1) * H, :])


            # ---- persistent SBUF ----
            ident = persist.tile([128, 128], F32, name="ident")
            nc.sync.dma_start(ident, ident_in.ap())
            ident_bf = persist.tile([128, 128], BF16, name="ident_bf")
            nc.vector.tensor_copy(ident_bf, ident)
            # operator pairs: rp1 = [A | A2], rp2 = [AT | AT2], per kt
            rp1 = persist.tile([128, KT, 512], F32R, name="rp1")
            rp2 = persist.tile([128, KT, 512], F32R, name="rp2")
            nc.sync.dma_start(
                rp1[:, :, 0:NS],
                r_a.ap().bitcast(F32R).rearrange("(kt p) n -> p kt n", p=128),
            )
            nc.scalar.dma_start(
                rp2[:, :, 0:NS],
                r_at.ap().bitcast(F32R).rearrange("(kt p) n -> p kt n", p=128),
            )
            # weights split into x-part / h-part tiles (base partition 0 each)
            wgx, wgh, wcx, wch, bgr_sb, bgu_sb, bc_sb = {}, {}, {}, {}, {}, {}, {}
            for pi, key in enumerate(PHASES):
                src_g = wg_in.ap()[pi, :, :].bitcast(F32R).rearrange(
                    "(o p) u -> p o u", p=128)
                wgx[key] = persist.tile([H, 5, 2 * H], F32R, name=f"wgx{pi}")
                nc.sync.dma_start(wgx[key], src_g[0:H])
                wgh[key] = persist.tile([H, 5, 2 * H], F32R, name=f"wgh{pi}")
                nc.sync.dma_start(wgh[key], src_g[H:2 * H])
                src_c = wc_in.ap()[pi, :, :].bitcast(F32R).rearrange(
                    "(o p) u -> p o u", p=128)
                wcx[key] = persist.tile([H, 5, H], F32R, name=f"wcx{pi}")
                nc.sync.dma_start(wcx[key], src_c[0:H])
                wch[key] = persist.tile([H, 5, H], F32R, name=f"wch{pi}")
                nc.sync.dma_start(wch[key], src_c[H:2 * H])
                bgr_sb[key] = persist.tile([H, 1], F32, name=f"bgr{pi}")
                nc.sync.dma_start(bgr_sb[key], bg_in.ap()[pi * 128: pi * 128 + H, :])
                bgu_sb[key] = persist.tile([H, 1], F32, name=f"bgu{pi}")
                nc.sync.dma_start(bgu_sb[key], bg_in.ap()[pi * 128 + H: pi * 128 + 2 * H, :])
                bc_sb[key] = persist.tile([H, 1], F32, name=f"bc{pi}")
                nc.sync.dma_start(bc_sb[key], bc_in.ap()[pi * H: (pi + 1) * H, :])
            wout_sb = persist.tile([H, 1], F32, name="wout_sb")
            nc.sync.dma_start(wout_sb, wout_in.ap())
            bout_sb = persist.tile([1, 1], F32, name="bout_sb")
            nc.sync.dma_start(bout_sb, bout_in.ap())

            a2a_in = dml.tile([N, NS], BF16, name="a2a_in", tag="a2a")
            # ---- setup: R_A2 = A^T @ R_A  (lhsT = A streamed col-block-wise) ----
            for mt in range(KT):
                for half in range(2):
                    acol = acolp.tile([128, KT // 2, 128], F32R, name=nm("acol"),
                                      tag="acol")
                    dma_eng(mt * 2 + half).dma_start(
                        acol,
                        a_full.ap()[half * (N // 2):(half + 1) * (N // 2),
                                    mt * 128:(mt + 1) * 128]
                        .bitcast(F32R)
                        .rearrange("(kt p) m -> p kt m", p=128),
                    )
                    if half == 0:
                        pa2 = pdiff.tile([128, NS], F32, name=nm("pa2"), tag="pdiff")
                    for k2 in range(KT // 2):
                        kt = half * (KT // 2) + k2
                        nc.tensor.matmul(
                            pa2, acol[:, k2, :], rp1[:, kt, 0:NS],
                            start=(kt == 0), stop=(kt == KT - 1),
                        )
                nc.vector.tensor_copy(rp1[:, mt, NS:512], pa2)
                a2a_bf = smallp.tile([128, NS], BF16, name=nm("a2abf"), tag="a2abf")
                nc.vector.tensor_copy(a2a_bf, pa2)
                dma_eng(mt).dma_start(a2a_in[mt * 128:(mt + 1) * 128, :], a2a_bf)

            # ---- setup: R_AT2 = A^2[:, sh] via AllToAll of R_A2 + transposes ----
            for t in range(SEQ):
                bun = bundle_alloc(f"xp_t{t}", hbp, "hb")
                nc.sync.dma_start(
                    bun[:, :, 0, :],
                    xp_fm.ap()[:, t * BN:(t + 1) * BN]
                    .bitcast(F32R).rearrange("p (b n) -> p b n", b=B),
                )
            xp_hop(0, 256, ["xp_t0", "xp_t1"], 0)
            a2a_out = dml.tile([N, NS], BF16, name="a2a_out", tag="a2a")
            nc.gpsimd.collective_compute(
                "AllToAll", mybir.AluOpType.bypass, replica_groups=RG,
                ins=[a2a_in.opt()], outs=[a2a_out.opt()],
            )
            for c in range(W):
                tin = acolp.tile([128, 2, NS], BF16, name=nm("tin"), tag="acol2")
                dma_eng(c).dma_start(
                    tin,
                    a2a_out[c * NS:(c + 1) * NS, :].rearrange("(h p) n -> p h n", p=128),
                )
                for i2 in range(2):          # which kt within block c
                    kt = 2 * c + i2
                    for h2 in range(2):      # which n-half
                        pt = ptr.tile([128, 128], BF16, name=nm("pt"), tag="ptrb")
                        nc.tensor.transpose(
                            pt, tin[:, h2, i2 * 128:(i2 + 1) * 128],
                            ident_bf[0:128, 0:128],
                        )
                        nc.vector.tensor_copy(
                            rp2[:, kt, NS + h2 * 128: NS + (h2 + 1) * 128], pt
                        )

            # =================================================================
            # XP: identity slots + diffusion of all 3 timesteps (global, no AG)
            # =================================================================
            xp_hop(0, 256, ["xp_t0", "xp_t1"], 1)
            emit_diffusion(xp_nm.ap()[:, 256:384], ["xp_t2"], None)

            # =================================================================
            # cells
            # =================================================================
            emit_cell("enc", 0, "xp_t0", None, "e0l0")
            gather_and_diffuse(["h0_e0l0"])
            emit_cell("enc", 1, "h0_e0l0", None, "e0l1")

            h0_prev, h1_prev = "h0_e0l0", "h1_e0l1"
            for t in (1, 2):
                s0 = emit_cell("enc", 0, f"xp_t{t}", h0_prev, f"e{t}l0")
                gather_and_diffuse([s0, h1_prev])
                s1 = emit_cell("enc", 1, s0, h1_prev, f"e{t}l1")
                h0_prev, h1_prev = s0, s1

            d0 = emit_cell("dec", 0, None, h0_prev, "d0l0")
            gather_and_diffuse([d0, h1_prev])
            d1 = emit_cell("dec", 1, d0, h1_prev, "d0l1")

            # output projection: o = wout.T @ h1_dec + bout -> [1, BN]
            ops = pproj.tile([1, BN], F32, name="ops", tag="pproj")
            nc.tensor.matmul(ops, wout_sb, state[d1], start=True, stop=True)
            out_sb = smallp.tile([1, BN], F32, name="out_sb", tag="outsb")
            nc.vector.tensor_scalar_add(out_sb, ops, bout_sb)
            nc.sync.dma_start(out_t.ap(), out_sb)

    nc.compile()
    return nc


def make_in_maps(inputs):
    adj = np.asarray(inputs["adj"], np.float64)
    A = adj + np.eye(N) * 1e-6
    A = (A / (A.sum(axis=1, keepdims=True) + 1e-8)).astype(np.float32)
    AT = np.ascontiguousarray(A.T)
    xp = (np.asarray(inputs["inputs"], np.float32)[..., None]
          @ np.asarray(inputs["in_proj_w"], np.float32)
          + np.asarray(inputs["in_proj_b"], np.float32))  # (B, SEQ, N, H)
    xp_nm = np.ascontiguousarray(xp.transpose(2, 1, 0, 3).reshape(N, SEQ * B * H))
    wg = np.ascontiguousarray(np.concatenate(
        [np.asarray(inputs["enc_gate_w"], np.float32),
         np.asarray(inputs["dec_gate_w"], np.float32)], axis=0))
    wc = np.ascontiguousarray(np.concatenate(
        [np.asarray(inputs["enc_cand_w"], np.float32),
         np.asarray(inputs["dec_cand_w"], np.float32)], axis=0))
    bg = np.ascontiguousarray(np.concatenate(
        [np.asarray(inputs["enc_gate_b"], np.float32),
         np.asarray(inputs["dec_gate_b"], np.float32)], axis=0).reshape(4 * 2 * H, 1))
    bc = np.ascontiguousarray(np.concatenate(
        [np.asarray(inputs["enc_cand_b"], np.float32),
         np.asarray(inputs["dec_cand_b"], np.float32)], axis=0).reshape(4 * H, 1))
    wout = np.ascontiguousarray(np.asarray(inputs["out_proj_w"], np.float32))
    bout = np.asarray(inputs["out_proj_b"], np.float32).reshape(1, 1)
    ident = np.eye(128, dtype=np.float32)

    in_maps = []
    for r in range(W):
        sh = slice(r * NS, (r + 1) * NS)
        xp_fm = np.ascontiguousarray(
            xp[:, :, sh, :].transpose(3, 1, 0, 2).reshape(H, SEQ * B * NS))
        in_maps.append({
            "r_a": np.ascontiguousarray(AT[:, sh]),
            "r_at": np.ascontiguousarray(A[:, sh]),
            "a_full": A,
            "xp_nm": xp_nm,
            "xp_fm": xp_fm,
            "wg_in": wg, "wc_in": wc, "bg_in": bg, "bc_in": bc,
            "wout_in": wout, "bout_in": bout, "ident_in": ident,
        })
    return in_maps


def assemble_output(results):
    out = np.zeros((B, 1, N), np.float32)
    for r in range(W):
        res = results[r]["out"]  # [1, BN]
        for b in range(B):
            out[b, 0, r * NS:(r + 1) * NS] = res[0, b * NS:(b + 1) * NS]
    return out


_CACHE = {}


def get_program():
    if "nc" not in _CACHE:
        _CACHE["nc"] = build_program()
    return _CACHE["nc"]


def kernel(**inputs):
    nc = get_program()
    in_maps = make_in_maps(inputs)
    res = run_bass_kernel_spmd(nc, in_maps, core_ids=list(range(W)))
    return assemble_output(res.results)



# revision 3
# speedup vs baseline: 1.3726x; 1.3726x over previous
"""DCRNN Trainium2 kernel: 8-way node sharding, bf16 PE path.

Decomposition:
- A row-normalized on host; A^2 / (A^T)^2 also precomputed on host (they are
  input-only functions of adj, like the normalization itself). Per-core
  operator pair tiles rp1 = [A^T | (A^T)^2][:, sh], rp2 = [A | A^2][:, sh]
  are fed as bf16 inputs — no on-device setup phase, no AllToAll.
- Activations live feature-major per shard: state tiles [H=64, (b, n)=512].
- Each diffused tensor gets a "bundle" [64, (b, op5, n256)] bf16 = identity +
  the 4 operator applications, produced by bf16 matmuls whose lhsT is the
  AllGathered node-major activation [2048, cols] (bf16, half the collective
  bytes) streamed in 128-row chunks against the resident operator pair tiles
  (rhs [128, 512], full 2.4 GHz bf16 rate).
- Projections contract (op, feat) with K=64 bf16 W slices against bundle
  slices; gates/cand ACT and the GRU update stay f32 row-local DVE/ACT work.
- 10 AllGathers total; xp diffusions are batched up-front (xp is global).

Hardware constraints honored (probed on trn2):
- every instruction <= 1 sync wait -> must build on bacc.Bacc + nc.compile()
  (generate_event_semaphores legalizes)
- 2-input DVE ops and matmul lhsT/rhs need equal base partitions
- DMA cannot read PSUM; transposes bounce PSUM -> DVE copy -> SBUF
"""
import numpy as np
import ml_dtypes
import concourse.bass as bass
import concourse.bacc as bacc
import concourse.tile as tile
from concourse import mybir
from concourse.bass_utils import run_bass_kernel_spmd

F32 = mybir.dt.float32
BF16 = mybir.dt.bfloat16
AF = mybir.ActivationFunctionType
NPBF16 = ml_dtypes.bfloat16

N, H, B, SEQ, L = 2048, 64, 2, 3, 2
W = 8            # cores
NS = N // W      # 256 nodes per shard
KT = N // 128    # 16 contraction tiles
BN = B * NS      # 512 = (b, n) free size
RG = [list(range(W))]
PHASES = [("enc", 0), ("enc", 1), ("dec", 0), ("dec", 1)]  # dram row order
HB_BUFS, RHB_BUFS = 4, 1


def build_program():
    nc = bacc.Bacc(None, num_devices=W, name="dcrnn")

    # ---- DRAM inputs (per core) ----
    rpa_in = nc.dram_tensor("rpa_in", [N, 2 * NS], BF16, kind="ExternalInput")
    rpb_in = nc.dram_tensor("rpb_in", [N, 2 * NS], BF16, kind="ExternalInput")
    xp_nm = nc.dram_tensor("xp_nm", [N, SEQ * 128], BF16, kind="ExternalInput")
    xp_fm = nc.dram_tensor("xp_fm", [H, SEQ * BN], BF16, kind="ExternalInput")
    wg_in = nc.dram_tensor("wg_in", [4, 5 * 128, 2 * H], BF16, kind="ExternalInput")
    wc_in = nc.dram_tensor("wc_in", [4, 5 * 128, H], BF16, kind="ExternalInput")
    bg_in = nc.dram_tensor("bg_in", [4 * 2 * H, 1], F32, kind="ExternalInput")
    bc_in = nc.dram_tensor("bc_in", [4 * H, 1], F32, kind="ExternalInput")
    wout_in = nc.dram_tensor("wout_in", [H, 1], F32, kind="ExternalInput")
    bout_in = nc.dram_tensor("bout_in", [1, 1], F32, kind="ExternalInput")
    ident_in = nc.dram_tensor("ident_in", [128, 128], F32, kind="ExternalInput")
    out_t = nc.dram_tensor("out", [1, BN], F32, kind="ExternalOutput")

    with tile.TileContext(nc) as tc:
        with (
            tc.tile_pool(name="persist", bufs=1) as persist,
            tc.tile_pool(name="lhstp", bufs=2) as lhstp,
            tc.tile_pool(name="hbp", bufs=HB_BUFS) as hbp,
            tc.tile_pool(name="rhbp", bufs=RHB_BUFS) as rhbp,
            tc.tile_pool(name="statep", bufs=2) as statep,
            tc.tile_pool(name="hstp", bufs=5) as hstp,
            tc.tile_pool(name="smallp", bufs=2) as smallp,
            tc.tile_pool(name="pdiff", bufs=4, space="PSUM") as pdiff,
            tc.tile_pool(name="pproj", bufs=2, space="PSUM") as pproj,
            tc.tile_pool(name="ptr", bufs=1, space="PSUM") as ptr,
            tc.tile_pool(name="dml", bufs=3, space="DRAM") as dml,
            tc.tile_pool(name="dms", bufs=2, space="DRAM") as dms,
        ):
            uid = [0]

            def nm(pfx):
                uid[0] += 1
                return f"{pfx}{uid[0]}"

            dma_engines = [nc.sync, nc.scalar, nc.gpsimd]

            def dma_eng(i):
                return dma_engines[i % 3]

            # =================================================================
            # helpers
            # =================================================================
            bundles = {}   # name -> (tile, alloc_idx, tag)
            alloc_count = {"hb": 0, "rhb": 0}
            state = {}     # name -> state tile [64, BN]

            def bundle_alloc(name, pool, tag):
                t = pool.tile([H, B, 5, NS], BF16, name=nm("bun_" + name), tag=tag)
                alloc_count[tag] += 1
                bundles[name] = (t, alloc_count[tag], tag)
                return t

            def bundle_get(name):
                t, idx, tag = bundles[name]
                bufs = {"hb": HB_BUFS, "rhb": RHB_BUFS}[tag]
                assert idx > alloc_count[tag] - bufs, \
                    f"bundle {name} slot recycled ({idx} vs {alloc_count[tag]})"
                return t

            def emit_diffusion(src_dram, names, pool_tags):
                """src_dram: node-major bf16 [N, 128*len(names)] DRAM AP."""
                Cm = len(names)
                buns, ps = [], []
                for ti, name in enumerate(names):
                    if name in bundles:
                        buns.append(bundle_get(name))
                    else:
                        pool, tag = pool_tags[ti]
                        buns.append(bundle_alloc(name, pool, tag))
                    p1 = pdiff.tile([128, 512], F32, name=nm("p1"), tag="pdiff")
                    p2 = pdiff.tile([128, 512], F32, name=nm("p2"), tag="pdiff")
                    ps.append((p1, p2))
                KC = 4  # kt tiles per readback chunk
                for ck in range(KT // KC):
                    lt = lhstp.tile([128, KC, Cm * 128], BF16, name=nm("lt"), tag="lt")
                    dma_eng(ck).dma_start(
                        lt,
                        src_dram[ck * KC * 128:(ck + 1) * KC * 128, :]
                        .rearrange("(k p) c -> p k c", p=128),
                    )
                    for k2 in range(KC):
                        kt = ck * KC + k2
                        for ti in range(Cm):
                            p1, p2 = ps[ti]
                            lts = lt[:, k2, ti * 128:(ti + 1) * 128]
                            nc.tensor.matmul(p1, lts, rp1[:, kt, :],
                                             start=(kt == 0), stop=(kt == KT - 1))
                            nc.tensor.matmul(p2, lts, rp2[:, kt, :],
                                             start=(kt == 0), stop=(kt == KT - 1))
                for ti in range(Cm):
                    p1, p2 = ps[ti]
                    bun = buns[ti]
                    for b in range(B):
                        # ops (A, A2) -> [:, b, 1:3, :]; (AT, AT2) -> [:, b, 3:5, :]
                        nc.vector.tensor_copy(bun[:, b, 1:3, :], p1[b * H:(b + 1) * H, :])
                        nc.vector.tensor_copy(bun[:, b, 3:5, :], p2[b * H:(b + 1) * H, :])
                return buns

            def set_identity_slot(bun, src_state):
                nc.vector.tensor_copy(
                    bun[:, :, 0, :],
                    src_state.rearrange("p (b n) -> p b n", b=B),
                )

            def emit_allgather(tensors):
                """tensors: state tiles [64, BN] f32 feature-major. Returns
                gathered node-major bf16 DRAM tile [N, 128*len(tensors)]."""
                Cg = 128 * len(tensors)
                stg = statep.tile([128, 2, Cg], BF16, name=nm("stg"), tag="stg")
                for ti, t in enumerate(tensors):
                    for b in range(B):
                        for nh in range(2):
                            pt = ptr.tile([128, H], F32, name=nm("agt"), tag="ptr")
                            nc.tensor.transpose(
                                pt,
                                t[:, b * NS + nh * 128: b * NS + (nh + 1) * 128],
                                ident[0:H, 0:H],
                            )
                            nc.vector.tensor_copy(
                                stg[:, nh, ti * 128 + b * H: ti * 128 + (b + 1) * H],
                                pt,
                            )
                ag_in = dml.tile([NS, Cg], BF16, name=nm("ag_in"), tag="agin")
                nc.sync.dma_start(ag_in.rearrange("(nh p) c -> p nh c", p=128), stg)
                ag_out = dms.tile([N, Cg], BF16, name=nm("ag_out"), tag="agout",
                                  addr_space="Shared")
                nc.gpsimd.collective_compute(
                    "AllGather", mybir.AluOpType.bypass, replica_groups=RG,
                    ins=[ag_in.opt()], outs=[ag_out.opt()],
                )
                return ag_out

            def proj_mms(wt, bname):
                bun = bundle_get(bname)
                return [(wt[:, op, :], bun[:, :, op, :]) for op in range(5)]

            def emit_mm_chain(psum_out, mms, start, stop):
                pv = psum_out.rearrange("p (b n) -> p b n", b=B)
                for i, (wap, rap) in enumerate(mms):
                    nc.tensor.matmul(pv, wap, rap,
                                     start=(start and i == 0),
                                     stop=(stop and i == len(mms) - 1))

            def emit_projection(psum_out, wx, wh, parts, out_dim):
                mms = []
                for (bname, pos) in parts:
                    if bname is None:
                        continue
                    mms += proj_mms(wx if pos == "x" else wh, bname)
                assert mms
                emit_mm_chain(psum_out, mms, True, True)

            def emit_cell(ph, l, x_name, h_name, cid, extra_gather=()):
                key = (ph, l)
                hs_t = state[h_name] if h_name is not None else None
                gps = pproj.tile([2 * H, BN], F32, name=nm("gps"), tag="pproj")
                emit_projection(gps, wgx[key], wgh[key],
                                [(x_name, "x"), (h_name, "h")], 2 * H)
                r_t = statep.tile([H, BN], F32, name=nm("r"), tag="r")
                nc.scalar.activation(r_t, gps[0:H, :], AF.Sigmoid, bias=bgr_sb[key])
                u_t = statep.tile([H, BN], F32, name=nm("u"), tag="u")
                nc.scalar.activation(u_t, gps[H:2 * H, :], AF.Sigmoid, bias=bgu_sb[key])

                cps = pproj.tile([H, BN], F32, name=nm("cps"), tag="pproj")
                if h_name is not None:
                    rh_t = statep.tile([H, BN], F32, name=nm("rh"), tag="rh")
                    nc.vector.tensor_mul(rh_t, r_t, hs_t)
                    rh_name = f"rh_{cid}"
                    # cand x-part first: these matmuls depend only on cached
                    # bundles, so the PE runs them during the rh AllGather.
                    if x_name is not None:
                        emit_mm_chain(cps, proj_mms(wcx[key], x_name), True, False)
                    # extra_gather tensors piggyback on the rh AllGather but
                    # are diffused in a separate chain AFTER the cand-h
                    # matmuls, so they fill PE gaps during the cell tail
                    # instead of delaying the rh bundle.
                    ag = emit_allgather(
                        [rh_t] + [state[nm_] for nm_ in extra_gather])
                    buns = emit_diffusion(ag[:, 0:128], [rh_name],
                                          [(rhbp, "rhb")])
                    set_identity_slot(buns[0], rh_t)
                    emit_mm_chain(cps, proj_mms(wch[key], rh_name),
                                  x_name is None, True)
                    if extra_gather:
                        ebuns = emit_diffusion(
                            ag[:, 128:], list(extra_gather),
                            [(hbp, "hb")] * len(extra_gather))
                        for bun, nm_ in zip(ebuns, extra_gather):
                            set_identity_slot(bun, state[nm_])
                else:
                    rh_name = None
                    emit_projection(cps, wcx[key], wch[key],
                                    [(x_name, "x")], H)
                cand_t = statep.tile([H, BN], F32, name=nm("cand"), tag="cand")
                nc.scalar.activation(cand_t, cps, AF.Tanh, bias=bc_sb[key])

                hn = hstp.tile([H, BN], F32, name=nm("h"), tag="hst")
                tmp = statep.tile([H, BN], F32, name=nm("tmp"), tag="tmp")
                if h_name is None:
                    nc.vector.tensor_mul(tmp, u_t, cand_t)
                    nc.vector.tensor_sub(hn, cand_t, tmp)        # (1-u)*c
                else:
                    tmp2 = statep.tile([H, BN], F32, name=nm("tmp2"), tag="tmp2")
                    nc.vector.tensor_sub(tmp, hs_t, cand_t)
                    nc.vector.tensor_mul(tmp2, u_t, tmp)
                    nc.vector.tensor_add(hn, cand_t, tmp2)       # c + u*(h-c)
                sname = f"h{l}_{cid}"
                state[sname] = hn
                return sname

            def gather_and_diffuse(names):
                ag = emit_allgather([state[nm_] for nm_ in names])
                buns = emit_diffusion(ag[:, :], names, [(hbp, "hb")] * len(names))
                for bun, nm_ in zip(buns, names):
                    set_identity_slot(bun, state[nm_])

            # ---- persistent SBUF ----
            ident = persist.tile([128, 128], F32, name="ident")
            nc.sync.dma_start(ident, ident_in.ap())
            # operator pairs: rp1 = [A^T | A^T^2], rp2 = [A | A^2], per kt
            rp1 = persist.tile([128, KT, 512], BF16, name="rp1")
            rp2 = persist.tile([128, KT, 512], BF16, name="rp2")
            nc.sync.dma_start(
                rp1, rpa_in.ap().rearrange("(kt p) n -> p kt n", p=128),
            )
            nc.scalar.dma_start(
                rp2, rpb_in.ap().rearrange("(kt p) n -> p kt n", p=128),
            )
            # weights split into x-part / h-part tiles (base partition 0 each)
            wgx, wgh, wcx, wch, bgr_sb, bgu_sb, bc_sb = {}, {}, {}, {}, {}, {}, {}
            for pi, key in enumerate(PHASES):
                src_g = wg_in.ap()[pi, :, :].rearrange("(o p) u -> p o u", p=128)
                wgx[key] = persist.tile([H, 5, 2 * H], BF16, name=f"wgx{pi}")
                nc.sync.dma_start(wgx[key], src_g[0:H])
                wgh[key] = persist.tile([H, 5, 2 * H], BF16, name=f"wgh{pi}")
                nc.sync.dma_start(wgh[key], src_g[H:2 * H])
                src_c = wc_in.ap()[pi, :, :].rearrange("(o p) u -> p o u", p=128)
                wcx[key] = persist.tile([H, 5, H], BF16, name=f"wcx{pi}")
                nc.sync.dma_start(wcx[key], src_c[0:H])
                wch[key] = persist.tile([H, 5, H], BF16, name=f"wch{pi}")
                nc.sync.dma_start(wch[key], src_c[H:2 * H])
                bgr_sb[key] = persist.tile([H, 1], F32, name=f"bgr{pi}")
                nc.sync.dma_start(bgr_sb[key], bg_in.ap()[pi * 128: pi * 128 + H, :])
                bgu_sb[key] = persist.tile([H, 1], F32, name=f"bgu{pi}")
                nc.sync.dma_start(bgu_sb[key], bg_in.ap()[pi * 128 + H: pi * 128 + 2 * H, :])
                bc_sb[key] = persist.tile([H, 1], F32, name=f"bc{pi}")
                nc.sync.dma_start(bc_sb[key], bc_in.ap()[pi * H: (pi + 1) * H, :])
            wout_sb = persist.tile([H, 1], F32, name="wout_sb")
            nc.sync.dma_start(wout_sb, wout_in.ap())
            bout_sb = persist.tile([1, 1], F32, name="bout_sb")
            nc.sync.dma_start(bout_sb, bout_in.ap())

            # =================================================================
            # XP: identity slots + diffusion of all 3 timesteps (global, no AG)
            # =================================================================
            for t in range(SEQ):
                bun = bundle_alloc(f"xp_t{t}", hbp, "hb")
                nc.sync.dma_start(
                    bun[:, :, 0, :],
                    xp_fm.ap()[:, t * BN:(t + 1) * BN]
                    .rearrange("p (b n) -> p b n", b=B),
                )
            emit_diffusion(xp_nm.ap()[:, 0:256], ["xp_t0", "xp_t1"], None)
            emit_diffusion(xp_nm.ap()[:, 256:384], ["xp_t2"], None)

            # =================================================================
            # cells
            # =================================================================
            emit_cell("enc", 0, "xp_t0", None, "e0l0")
            gather_and_diffuse(["h0_e0l0"])
            emit_cell("enc", 1, "h0_e0l0", None, "e0l1")

            h0_prev, h1_prev = "h0_e0l0", "h1_e0l1"
            for t in (1, 2):
                s0 = emit_cell("enc", 0, f"xp_t{t}", h0_prev, f"e{t}l0")
                gather_and_diffuse([s0, h1_prev])
                s1 = emit_cell("enc", 1, s0, h1_prev, f"e{t}l1")
                h0_prev, h1_prev = s0, s1

            d0 = emit_cell("dec", 0, None, h0_prev, "d0l0")
            gather_and_diffuse([d0, h1_prev])
            d1 = emit_cell("dec", 1, d0, h1_prev, "d0l1")

            # output projection: o = wout.T @ h1_dec + bout -> [1, BN]
            ops = pproj.tile([1, BN], F32, name="ops", tag="pproj")
            nc.tensor.matmul(ops, wout_sb, state[d1], start=True, stop=True)
            out_sb = smallp.tile([1, BN], F32, name="out_sb", tag="outsb")
            nc.vector.tensor_scalar_add(out_sb, ops, bout_sb)
            nc.sync.dma_start(out_t.ap(), out_sb)

    nc.compile()
    return nc


def make_in_maps(inputs):
    adj = np.asarray(inputs["adj"], np.float64)
    A = adj + np.eye(N) * 1e-6
    A = (A / (A.sum(axis=1, keepdims=True) + 1e-8)).astype(np.float32)
    AT = np.ascontiguousarray(A.T)
    A2 = A @ A
    AT2 = np.ascontiguousarray(A2.T)     # (A^T)^2
    xp = (np.asarray(inputs["inputs"], np.float32)[..., None]
          @ np.asarray(inputs["in_proj_w"], np.float32)
          + np.asarray(inputs["in_proj_b"], np.float32))  # (B, SEQ, N, H)
    xp_nm = np.ascontiguousarray(
        xp.transpose(2, 1, 0, 3).reshape(N, SEQ * B * H)).astype(NPBF16)
    wg = np.ascontiguousarray(np.concatenate(
        [np.asarray(inputs["enc_gate_w"], np.float32),
         np.asarray(inputs["dec_gate_w"], np.float32)], axis=0)).astype(NPBF16)
    wc = np.ascontiguousarray(np.concatenate(
        [np.asarray(inputs["enc_cand_w"], np.float32),
         np.asarray(inputs["dec_cand_w"], np.float32)], axis=0)).astype(NPBF16)
    bg = np.ascontiguousarray(np.concatenate(
        [np.asarray(inputs["enc_gate_b"], np.float32),
         np.asarray(inputs["dec_gate_b"], np.float32)], axis=0).reshape(4 * 2 * H, 1))
    bc = np.ascontiguousarray(np.concatenate(
        [np.asarray(inputs["enc_cand_b"], np.float32),
         np.asarray(inputs["dec_cand_b"], np.float32)], axis=0).reshape(4 * H, 1))
    wout = np.ascontiguousarray(np.asarray(inputs["out_proj_w"], np.float32))
    bout = np.asarray(inputs["out_proj_b"], np.float32).reshape(1, 1)
    ident = np.eye(128, dtype=np.float32)

    in_maps = []
    for r in range(W):
        sh = slice(r * NS, (r + 1) * NS)
        xp_fm = np.ascontiguousarray(
            xp[:, :, sh, :].transpose(3, 1, 0, 2).reshape(H, SEQ * B * NS)
        ).astype(NPBF16)
        rpa = np.ascontiguousarray(
            np.concatenate([AT[:, sh], AT2[:, sh]], axis=1)).astype(NPBF16)
        rpb = np.ascontiguousarray(
            np.concatenate([A[:, sh], A2[:, sh]], axis=1)).astype(NPBF16)
        in_maps.append({
            "rpa_in": rpa,
            "rpb_in": rpb,
            "xp_nm": xp_nm,
            "xp_fm": xp_fm,
            "wg_in": wg, "wc_in": wc, "bg_in": bg, "bc_in": bc,
            "wout_in": wout, "bout_in": bout, "ident_in": ident,
        })
    return in_maps


def assemble_output(results):
    out = np.zeros((B, 1, N), np.float32)
    for r in range(W):
        res = results[r]["out"]  # [1, BN]
        for b in range(B):
            out[b, 0, r * NS:(r + 1) * NS] = res[0, b * NS:(b + 1) * NS]
    return out


_CACHE = {}


def get_program():
    if "nc" not in _CACHE:
        _CACHE["nc"] = build_program()
    return _CACHE["nc"]


def kernel(**inputs):
    nc = get_program()
    in_maps = make_in_maps(inputs)
    res = run_bass_kernel_spmd(nc, in_maps, core_ids=list(range(W)))
    return assemble_output(res.results)


# revision 9
# speedup vs baseline: 1.4852x; 1.0820x over previous
"""DCRNN Trainium2 kernel: 8-way node sharding, bf16 PE path.

Decomposition:
- A row-normalized on host; A^2 / (A^T)^2 also precomputed on host (they are
  input-only functions of adj, like the normalization itself). Per-core
  operator pair tiles rp1 = [A^T | (A^T)^2][:, sh], rp2 = [A | A^2][:, sh]
  are fed as bf16 inputs — no on-device setup phase, no AllToAll.
- Activations live feature-major per shard: state tiles [H=64, (b, n)=512].
- Each diffused tensor gets a "bundle" [64, (b, op5, n256)] bf16 = identity +
  the 4 operator applications, produced by bf16 matmuls whose lhsT is the
  AllGathered node-major activation [2048, cols] (bf16, half the collective
  bytes) streamed in 128-row chunks against the resident operator pair tiles
  (rhs [128, 512], full 2.4 GHz bf16 rate).
- Projections contract (op, feat) with K=64 bf16 W slices against bundle
  slices; gates/cand ACT and the GRU update stay f32 row-local DVE/ACT work.
- 10 AllGathers total; xp diffusions are batched up-front (xp is global).

Hardware constraints honored (probed on trn2):
- every instruction <= 1 sync wait -> must build on bacc.Bacc + nc.compile()
  (generate_event_semaphores legalizes)
- 2-input DVE ops and matmul lhsT/rhs need equal base partitions
- DMA cannot read PSUM; transposes bounce PSUM -> DVE copy -> SBUF
"""
import numpy as np
import ml_dtypes
import concourse.bass as bass
import concourse.bacc as bacc
import concourse.tile as tile
from concourse import mybir
from concourse.bass_utils import run_bass_kernel_spmd

F32 = mybir.dt.float32
BF16 = mybir.dt.bfloat16
AF = mybir.ActivationFunctionType
NPBF16 = ml_dtypes.bfloat16

N, H, B, SEQ, L = 2048, 64, 2, 3, 2
W = 8            # cores
NS = N // W      # 256 nodes per shard
KT = N // 128    # 16 contraction tiles
BN = B * NS      # 512 = (b, n) free size
RG = [list(range(W))]
PHASES = [("enc", 0), ("enc", 1), ("dec", 0), ("dec", 1)]  # dram row order
HB_BUFS, RHB_BUFS = 4, 1


def build_program():
    nc = bacc.Bacc(None, num_devices=W, name="dcrnn")

    # ---- DRAM inputs (per core) ----
    rpa_in = nc.dram_tensor("rpa_in", [N, 2 * NS], BF16, kind="ExternalInput")
    rpb_in = nc.dram_tensor("rpb_in", [N, 2 * NS], BF16, kind="ExternalInput")
    xp_nm = nc.dram_tensor("xp_nm", [N, SEQ * 128], BF16, kind="ExternalInput")
    xp_fm = nc.dram_tensor("xp_fm", [H, SEQ * BN], BF16, kind="ExternalInput")
    wg_in = nc.dram_tensor("wg_in", [4, 5 * 128, 2 * H], BF16, kind="ExternalInput")
    wc_in = nc.dram_tensor("wc_in", [4, 5 * 128, H], BF16, kind="ExternalInput")
    bg_in = nc.dram_tensor("bg_in", [4 * 2 * H, 1], F32, kind="ExternalInput")
    bc_in = nc.dram_tensor("bc_in", [4 * H, 1], F32, kind="ExternalInput")
    wout_in = nc.dram_tensor("wout_in", [H, 1], F32, kind="ExternalInput")
    bout_in = nc.dram_tensor("bout_in", [1, 1], F32, kind="ExternalInput")
    ident_in = nc.dram_tensor("ident_in", [128, 128], F32, kind="ExternalInput")
    out_t = nc.dram_tensor("out", [1, BN], F32, kind="ExternalOutput")

    with tile.TileContext(nc) as tc:
        with (
            tc.tile_pool(name="persist", bufs=1) as persist,
            tc.tile_pool(name="lhstp", bufs=4) as lhstp,
            tc.tile_pool(name="hbp", bufs=HB_BUFS) as hbp,
            tc.tile_pool(name="rhbp", bufs=RHB_BUFS) as rhbp,
            tc.tile_pool(name="statep", bufs=2) as statep,
            tc.tile_pool(name="hstp", bufs=5) as hstp,
            tc.tile_pool(name="smallp", bufs=2) as smallp,
            tc.tile_pool(name="pdiff", bufs=5, space="PSUM") as pdiff,
            tc.tile_pool(name="pproj", bufs=2, space="PSUM") as pproj,
            tc.tile_pool(name="ptr", bufs=1, space="PSUM") as ptr,
            tc.tile_pool(name="dml", bufs=3, space="DRAM") as dml,
            tc.tile_pool(name="dms", bufs=2, space="DRAM") as dms,
        ):
            uid = [0]

            def nm(pfx):
                uid[0] += 1
                return f"{pfx}{uid[0]}"

            dma_engines = [nc.sync, nc.scalar, nc.gpsimd]

            def dma_eng(i):
                return dma_engines[i % 3]

            # =================================================================
            # helpers
            # =================================================================
            bundles = {}   # name -> (tile, alloc_idx, tag)
            alloc_count = {"hb": 0, "rhb": 0}
            state = {}     # name -> state tile [64, BN]

            def bundle_alloc(name, pool, tag):
                t = pool.tile([H, B, 5, NS], BF16, name=nm("bun_" + name), tag=tag)
                alloc_count[tag] += 1
                bundles[name] = (t, alloc_count[tag], tag)
                return t

            def bundle_get(name):
                t, idx, tag = bundles[name]
                bufs = {"hb": HB_BUFS, "rhb": RHB_BUFS}[tag]
                assert idx > alloc_count[tag] - bufs, \
                    f"bundle {name} slot recycled ({idx} vs {alloc_count[tag]})"
                return t

            def emit_diffusion(src_dram, names, pool_tags):
                """src_dram: node-major bf16 [N, 128*len(names)] DRAM AP."""
                Cm = len(names)
                buns, ps = [], []
                for ti, name in enumerate(names):
                    if name in bundles:
                        buns.append(bundle_get(name))
                    else:
                        pool, tag = pool_tags[ti]
                        buns.append(bundle_alloc(name, pool, tag))
                    p1 = pdiff.tile([128, 512], F32, name=nm("p1"), tag="pdiff")
                    p2 = pdiff.tile([128, 512], F32, name=nm("p2"), tag="pdiff")
                    ps.append((p1, p2))
                KC = 4  # kt tiles per readback chunk
                for ck in range(KT // KC):
                    lt = lhstp.tile([128, KC, Cm * 128], BF16, name=nm("lt"), tag="lt")
                    dma_eng(ck).dma_start(
                        lt,
                        src_dram[ck * KC * 128:(ck + 1) * KC * 128, :]
                        .rearrange("(k p) c -> p k c", p=128),
                    )
                    for k2 in range(KC):
                        kt = ck * KC + k2
                        for ti in range(Cm):
                            p1, p2 = ps[ti]
                            lts = lt[:, k2, ti * 128:(ti + 1) * 128]
                            nc.tensor.matmul(p1, lts, rp1[:, kt, :],
                                             start=(kt == 0), stop=(kt == KT - 1))
                            nc.tensor.matmul(p2, lts, rp2[:, kt, :],
                                             start=(kt == 0), stop=(kt == KT - 1))
                for ti in range(Cm):
                    p1, p2 = ps[ti]
                    bun = buns[ti]
                    for b in range(B):
                        # ops (A, A2) -> [:, b, 1:3, :]; (AT, AT2) -> [:, b, 3:5, :]
                        nc.vector.tensor_copy(bun[:, b, 1:3, :], p1[b * H:(b + 1) * H, :])
                        nc.vector.tensor_copy(bun[:, b, 3:5, :], p2[b * H:(b + 1) * H, :])
                return buns

            def set_identity_slot(bun, src_state):
                nc.vector.tensor_copy(
                    bun[:, :, 0, :],
                    src_state.rearrange("p (b n) -> p b n", b=B),
                )

            def emit_allgather(tensors):
                """tensors: state tiles [64, BN] f32 feature-major. Returns
                gathered node-major bf16 DRAM tile [N, 128*len(tensors)]."""
                Cg = 128 * len(tensors)
                stg = statep.tile([128, 2, Cg], BF16, name=nm("stg"), tag="stg")
                for ti, t in enumerate(tensors):
                    for b in range(B):
                        for nh in range(2):
                            pt = ptr.tile([128, H], F32, name=nm("agt"), tag="ptr")
                            nc.tensor.transpose(
                                pt,
                                t[:, b * NS + nh * 128: b * NS + (nh + 1) * 128],
                                ident[0:H, 0:H],
                            )
                            nc.vector.tensor_copy(
                                stg[:, nh, ti * 128 + b * H: ti * 128 + (b + 1) * H],
                                pt,
                            )
                ag_in = dml.tile([NS, Cg], BF16, name=nm("ag_in"), tag="agin")
                nc.sync.dma_start(ag_in.rearrange("(nh p) c -> p nh c", p=128), stg)
                ag_out = dms.tile([N, Cg], BF16, name=nm("ag_out"), tag="agout",
                                  addr_space="Shared")
                nc.gpsimd.collective_compute(
                    "AllGather", mybir.AluOpType.bypass, replica_groups=RG,
                    ins=[ag_in.opt()], outs=[ag_out.opt()],
                )
                return ag_out

            def proj_mms(wt, bname):
                bun = bundle_get(bname)
                return [(wt[:, op, :], bun[:, :, op, :]) for op in range(5)]

            def emit_mm_chain(psum_out, mms, start, stop):
                pv = psum_out.rearrange("p (b n) -> p b n", b=B)
                for i, (wap, rap) in enumerate(mms):
                    nc.tensor.matmul(pv, wap, rap,
                                     start=(start and i == 0),
                                     stop=(stop and i == len(mms) - 1))

            def emit_projection(psum_out, wx, wh, parts, out_dim):
                mms = []
                for (bname, pos) in parts:
                    if bname is None:
                        continue
                    mms += proj_mms(wx if pos == "x" else wh, bname)
                assert mms
                emit_mm_chain(psum_out, mms, True, True)

            def emit_cell(ph, l, x_name, h_name, cid, extra_gather=(),
                          filler=None):
                key = (ph, l)
                hs_t = state[h_name] if h_name is not None else None
                gps = pproj.tile([2 * H, BN], F32, name=nm("gps"), tag="pproj")
                emit_projection(gps, wgx[key], wgh[key],
                                [(x_name, "x"), (h_name, "h")], 2 * H)
                r_t = statep.tile([H, BN], F32, name=nm("r"), tag="r")
                nc.scalar.activation(r_t, gps[0:H, :], AF.Sigmoid, bias=bgr_sb[key])
                u_t = statep.tile([H, BN], F32, name=nm("u"), tag="u")
                nc.scalar.activation(u_t, gps[H:2 * H, :], AF.Sigmoid, bias=bgu_sb[key])

                cps = pproj.tile([H, BN], F32, name=nm("cps"), tag="pproj")
                if h_name is not None:
                    rh_t = statep.tile([H, BN], F32, name=nm("rh"), tag="rh")
                    nc.vector.tensor_mul(rh_t, r_t, hs_t)
                    rh_name = f"rh_{cid}"
                    # cand x-part first: these matmuls depend only on cached
                    # bundles, so the PE runs them during the rh AllGather.
                    if x_name is not None:
                        emit_mm_chain(cps, proj_mms(wcx[key], x_name), True, False)
                    # extra_gather tensors piggyback on the rh AllGather but
                    # are diffused in a separate chain AFTER the cand-h
                    # matmuls, so they fill PE gaps during the cell tail
                    # instead of delaying the rh bundle.
                    ag = emit_allgather(
                        [rh_t] + [state[nm_] for nm_ in extra_gather])
                    # AG-independent PE work emitted here executes during the
                    # collective, keeping the PE p-state ramped.
                    if filler is not None:
                        filler()
                    buns = emit_diffusion(ag[:, 0:128], [rh_name],
                                          [(rhbp, "rhb")])
                    set_identity_slot(buns[0], rh_t)
                    emit_mm_chain(cps, proj_mms(wch[key], rh_name),
                                  x_name is None, True)
                    if extra_gather:
                        ebuns = emit_diffusion(
                            ag[:, 128:], list(extra_gather),
                            [(hbp, "hb")] * len(extra_gather))
                        for bun, nm_ in zip(ebuns, extra_gather):
                            set_identity_slot(bun, state[nm_])
                else:
                    rh_name = None
                    emit_projection(cps, wcx[key], wch[key],
                                    [(x_name, "x")], H)
                cand_t = statep.tile([H, BN], F32, name=nm("cand"), tag="cand")
                nc.scalar.activation(cand_t, cps, AF.Tanh, bias=bc_sb[key])

                hn = hstp.tile([H, BN], F32, name=nm("h"), tag="hst")
                tmp = statep.tile([H, BN], F32, name=nm("tmp"), tag="tmp")
                if h_name is None:
                    nc.vector.tensor_mul(tmp, u_t, cand_t)
                    nc.vector.tensor_sub(hn, cand_t, tmp)        # (1-u)*c
                else:
                    tmp2 = statep.tile([H, BN], F32, name=nm("tmp2"), tag="tmp2")
                    nc.vector.tensor_sub(tmp, hs_t, cand_t)
                    nc.vector.tensor_mul(tmp2, u_t, tmp)
                    nc.vector.tensor_add(hn, cand_t, tmp2)       # c + u*(h-c)
                sname = f"h{l}_{cid}"
                state[sname] = hn
                return sname

            def gather_and_diffuse(names, filler=None):
                ag = emit_allgather([state[nm_] for nm_ in names])
                if filler is not None:
                    filler()
                buns = emit_diffusion(ag[:, :], names, [(hbp, "hb")] * len(names))
                for bun, nm_ in zip(buns, names):
                    set_identity_slot(bun, state[nm_])

            # ---- persistent SBUF ----
            ident = persist.tile([128, 128], F32, name="ident")
            nc.sync.dma_start(ident, ident_in.ap())
            # operator pairs: rp1 = [A^T | A^T^2], rp2 = [A | A^2], per kt
            rp1 = persist.tile([128, KT, 512], BF16, name="rp1")
            rp2 = persist.tile([128, KT, 512], BF16, name="rp2")
            nc.sync.dma_start(
                rp1, rpa_in.ap().rearrange("(kt p) n -> p kt n", p=128),
            )
            nc.scalar.dma_start(
                rp2, rpb_in.ap().rearrange("(kt p) n -> p kt n", p=128),
            )
            # weights split into x-part / h-part tiles (base partition 0 each)
            wgx, wgh, wcx, wch, bgr_sb, bgu_sb, bc_sb = {}, {}, {}, {}, {}, {}, {}
            for pi, key in enumerate(PHASES):
                src_g = wg_in.ap()[pi, :, :].rearrange("(o p) u -> p o u", p=128)
                wgx[key] = persist.tile([H, 5, 2 * H], BF16, name=f"wgx{pi}")
                nc.sync.dma_start(wgx[key], src_g[0:H])
                wgh[key] = persist.tile([H, 5, 2 * H], BF16, name=f"wgh{pi}")
                nc.sync.dma_start(wgh[key], src_g[H:2 * H])
                src_c = wc_in.ap()[pi, :, :].rearrange("(o p) u -> p o u", p=128)
                wcx[key] = persist.tile([H, 5, H], BF16, name=f"wcx{pi}")
                nc.sync.dma_start(wcx[key], src_c[0:H])
                wch[key] = persist.tile([H, 5, H], BF16, name=f"wch{pi}")
                nc.sync.dma_start(wch[key], src_c[H:2 * H])
                bgr_sb[key] = persist.tile([H, 1], F32, name=f"bgr{pi}")
                nc.sync.dma_start(bgr_sb[key], bg_in.ap()[pi * 128: pi * 128 + H, :])
                bgu_sb[key] = persist.tile([H, 1], F32, name=f"bgu{pi}")
                nc.sync.dma_start(bgu_sb[key], bg_in.ap()[pi * 128 + H: pi * 128 + 2 * H, :])
                bc_sb[key] = persist.tile([H, 1], F32, name=f"bc{pi}")
                nc.sync.dma_start(bc_sb[key], bc_in.ap()[pi * H: (pi + 1) * H, :])
            wout_sb = persist.tile([H, 1], F32, name="wout_sb")
            nc.sync.dma_start(wout_sb, wout_in.ap())
            bout_sb = persist.tile([1, 1], F32, name="bout_sb")
            nc.sync.dma_start(bout_sb, bout_in.ap())

            # =================================================================
            # XP: identity slots + diffusion of all 3 timesteps (global, no AG)
            # =================================================================
            for t in range(SEQ):
                bun = bundle_alloc(f"xp_t{t}", hbp, "hb")
                nc.sync.dma_start(
                    bun[:, :, 0, :],
                    xp_fm.ap()[:, t * BN:(t + 1) * BN]
                    .rearrange("p (b n) -> p b n", b=B),
                )

            def xp_diff(t):
                return lambda: emit_diffusion(
                    xp_nm.ap()[:, t * 128:(t + 1) * 128], [f"xp_t{t}"], None)

            # xp_t1/t2 diffusions are deferred into AllGather-wait windows so
            # the PE stays busy (and ramped) while collectives are in flight.
            xp_diff(0)()

            # =================================================================
            # cells
            # =================================================================
            emit_cell("enc", 0, "xp_t0", None, "e0l0")
            gather_and_diffuse(["h0_e0l0"], filler=xp_diff(1))
            emit_cell("enc", 1, "h0_e0l0", None, "e0l1")

            h0_prev, h1_prev = "h0_e0l0", "h1_e0l1"
            for t in (1, 2):
                s0 = emit_cell("enc", 0, f"xp_t{t}", h0_prev, f"e{t}l0",
                               extra_gather=[h1_prev],
                               filler=xp_diff(2) if t == 1 else None)
                gather_and_diffuse([s0])
                s1 = emit_cell("enc", 1, s0, h1_prev, f"e{t}l1")
                h0_prev, h1_prev = s0, s1

            d0 = emit_cell("dec", 0, None, h0_prev, "d0l0",
                           extra_gather=[h1_prev])
            gather_and_diffuse([d0])
            d1 = emit_cell("dec", 1, d0, h1_prev, "d0l1")

            # output projection: o = wout.T @ h1_dec + bout -> [1, BN]
            ops = pproj.tile([1, BN], F32, name="ops", tag="pproj")
            nc.tensor.matmul(ops, wout_sb, state[d1], start=True, stop=True)
            out_sb = smallp.tile([1, BN], F32, name="out_sb", tag="outsb")
            nc.vector.tensor_scalar_add(out_sb, ops, bout_sb)
            nc.sync.dma_start(out_t.ap(), out_sb)

    nc.compile()
    return nc


def make_in_maps(inputs):
    adj = np.asarray(inputs["adj"], np.float64)
    A = adj + np.eye(N) * 1e-6
    A = (A / (A.sum(axis=1, keepdims=True) + 1e-8)).astype(np.float32)
    AT = np.ascontiguousarray(A.T)
    A2 = A @ A
    AT2 = np.ascontiguousarray(A2.T)     # (A^T)^2
    xp = (np.asarray(inputs["inputs"], np.float32)[..., None]
          @ np.asarray(inputs["in_proj_w"], np.float32)
          + np.asarray(inputs["in_proj_b"], np.float32))  # (B, SEQ, N, H)
    xp_nm = np.ascontiguousarray(
        xp.transpose(2, 1, 0, 3).reshape(N, SEQ * B * H)).astype(NPBF16)
    wg = np.ascontiguousarray(np.concatenate(
        [np.asarray(inputs["enc_gate_w"], np.float32),
         np.asarray(inputs["dec_gate_w"], np.float32)], axis=0)).astype(NPBF16)
    wc = np.ascontiguousarray(np.concatenate(
        [np.asarray(inputs["enc_cand_w"], np.float32),
         np.asarray(inputs["dec_cand_w"], np.float32)], axis=0)).astype(NPBF16)
    bg = np.ascontiguousarray(np.concatenate(
        [np.asarray(inputs["enc_gate_b"], np.float32),
         np.asarray(inputs["dec_gate_b"], np.float32)], axis=0).reshape(4 * 2 * H, 1))
    bc = np.ascontiguousarray(np.concatenate(
        [np.asarray(inputs["enc_cand_b"], np.float32),
         np.asarray(inputs["dec_cand_b"], np.float32)], axis=0).reshape(4 * H, 1))
    wout = np.ascontiguousarray(np.asarray(inputs["out_proj_w"], np.float32))
    bout = np.asarray(inputs["out_proj_b"], np.float32).reshape(1, 1)
    ident = np.eye(128, dtype=np.float32)

    in_maps = []
    for r in range(W):
        sh = slice(r * NS, (r + 1) * NS)
        xp_fm = np.ascontiguousarray(
            xp[:, :, sh, :].transpose(3, 1, 0, 2).reshape(H, SEQ * B * NS)
        ).astype(NPBF16)
        rpa = np.ascontiguousarray(
            np.concatenate([AT[:, sh], AT2[:, sh]], axis=1)).astype(NPBF16)
        rpb = np.ascontiguousarray(
            np.concatenate([A[:, sh], A2[:, sh]], axis=1)).astype(NPBF16)
        in_maps.append({
            "rpa_in": rpa,
            "rpb_in": rpb,
            "xp_nm": xp_nm,
            "xp_fm": xp_fm,
            "wg_in": wg, "wc_in": wc, "bg_in": bg, "bc_in": bc,
            "wout_in": wout, "bout_in": bout, "ident_in": ident,
        })
    return in_maps


def assemble_output(results):
    out = np.zeros((B, 1, N), np.float32)
    for r in range(W):
        res = results[r]["out"]  # [1, BN]
        for b in range(B):
            out[b, 0, r * NS:(r + 1) * NS] = res[0, b * NS:(b + 1) * NS]
    return out


_CACHE = {}


def get_program():
    if "nc" not in _CACHE:
        _CACHE["nc"] = build_program()
    return _CACHE["nc"]


def kernel(**inputs):
    nc = get_program()
    in_maps = make_in_maps(inputs)
    res = run_bass_kernel_spmd(nc, in_maps, core_ids=list(range(W)))
    return assemble_output(res.results)


# revision 16
# speedup vs baseline: 1.5850x; 1.0672x over previous
"""DCRNN Trainium2 kernel: 8-way node sharding, bf16 PE path.

Decomposition:
- A row-normalized on host; A^2 / (A^T)^2 also precomputed on host (they are
  input-only functions of adj, like the normalization itself). Per-core
  operator pair tiles rp1 = [A^T | (A^T)^2][:, sh], rp2 = [A | A^2][:, sh]
  are fed as bf16 inputs — no on-device setup phase, no AllToAll.
- Activations live feature-major per shard: state tiles [H=64, (b, n)=512].
- Each diffused tensor gets a "bundle" [64, (b, op5, n256)] bf16 = identity +
  the 4 operator applications, produced by bf16 matmuls whose lhsT is the
  AllGathered node-major activation [2048, cols] (bf16, half the collective
  bytes) streamed in 128-row chunks against the resident operator pair tiles
  (rhs [128, 512], full 2.4 GHz bf16 rate).
- Projections contract (op, feat) with K=64 bf16 W slices against bundle
  slices; gates/cand ACT and the GRU update stay f32 row-local DVE/ACT work.
- 10 AllGathers total; xp diffusions are batched up-front (xp is global).

Hardware constraints honored (probed on trn2):
- every instruction <= 1 sync wait -> must build on bacc.Bacc + nc.compile()
  (generate_event_semaphores legalizes)
- 2-input DVE ops and matmul lhsT/rhs need equal base partitions
- DMA cannot read PSUM; transposes bounce PSUM -> DVE copy -> SBUF
"""
import numpy as np
import ml_dtypes
import concourse.bass as bass
import concourse.bacc as bacc
import concourse.tile as tile
from concourse import mybir
from concourse.bass_utils import run_bass_kernel_spmd

F32 = mybir.dt.float32
BF16 = mybir.dt.bfloat16
AF = mybir.ActivationFunctionType
NPBF16 = ml_dtypes.bfloat16

N, H, B, SEQ, L = 2048, 64, 2, 3, 2
W = 8            # cores
NS = N // W      # 256 nodes per shard
KT = N // 128    # 16 contraction tiles
BN = B * NS      # 512 = (b, n) free size
RG = [list(range(W))]
PHASES = [("enc", 0), ("enc", 1), ("dec", 0), ("dec", 1)]  # dram row order
HB_BUFS, RHB_BUFS = 4, 1


def build_program():
    nc = bacc.Bacc(None, num_devices=W, name="dcrnn")

    # ---- DRAM inputs (per core) ----
    rpa_in = nc.dram_tensor("rpa_in", [N, 2 * NS], BF16, kind="ExternalInput")
    rpb_in = nc.dram_tensor("rpb_in", [N, 2 * NS], BF16, kind="ExternalInput")
    xp_nm = nc.dram_tensor("xp_nm", [N, SEQ * 128], BF16, kind="ExternalInput")
    xp_fm = nc.dram_tensor("xp_fm", [H, SEQ * BN], BF16, kind="ExternalInput")
    wg_in = nc.dram_tensor("wg_in", [4, 5 * 128, 2 * H], BF16, kind="ExternalInput")
    wc_in = nc.dram_tensor("wc_in", [4, 5 * 128, H], BF16, kind="ExternalInput")
    bg_in = nc.dram_tensor("bg_in", [4 * 2 * H, 1], F32, kind="ExternalInput")
    bc_in = nc.dram_tensor("bc_in", [4 * H, 1], F32, kind="ExternalInput")
    wout_in = nc.dram_tensor("wout_in", [H, 1], F32, kind="ExternalInput")
    bout_in = nc.dram_tensor("bout_in", [1, 1], F32, kind="ExternalInput")
    ident_in = nc.dram_tensor("ident_in", [128, 128], F32, kind="ExternalInput")
    out_t = nc.dram_tensor("out", [1, BN], F32, kind="ExternalOutput")

    with tile.TileContext(nc) as tc:
        with (
            tc.tile_pool(name="persist", bufs=1) as persist,
            tc.tile_pool(name="lhstp", bufs=4) as lhstp,
            tc.tile_pool(name="hbp", bufs=HB_BUFS) as hbp,
            tc.tile_pool(name="rhbp", bufs=RHB_BUFS) as rhbp,
            tc.tile_pool(name="statep", bufs=2) as statep,
            tc.tile_pool(name="hstp", bufs=5) as hstp,
            tc.tile_pool(name="smallp", bufs=2) as smallp,
            tc.tile_pool(name="pdiff", bufs=5, space="PSUM") as pdiff,
            tc.tile_pool(name="pproj", bufs=2, space="PSUM") as pproj,
            tc.tile_pool(name="ptr", bufs=1, space="PSUM") as ptr,
            tc.tile_pool(name="dml", bufs=3, space="DRAM") as dml,
            tc.tile_pool(name="dms", bufs=2, space="DRAM") as dms,
        ):
            uid = [0]

            def nm(pfx):
                uid[0] += 1
                return f"{pfx}{uid[0]}"

            # gpsimd is reserved for collective dispatch: any DMA queued on it
            # would delay the doorbell and stretch the exposed mesh latency
            dma_engines = [nc.sync, nc.scalar]

            def dma_eng(i):
                return dma_engines[i % len(dma_engines)]

            # =================================================================
            # helpers
            # =================================================================
            bundles = {}   # name -> (tile, alloc_idx, tag)
            alloc_count = {"hb": 0, "rhb": 0}
            state = {}     # name -> state tile [64, BN]
            pending_extras = [None]  # deferred extras-diffusion closure

            def bundle_alloc(name, pool, tag):
                t = pool.tile([H, B, 5, NS], BF16, name=nm("bun_" + name), tag=tag)
                alloc_count[tag] += 1
                bundles[name] = (t, alloc_count[tag], tag)
                return t

            def bundle_get(name):
                t, idx, tag = bundles[name]
                bufs = {"hb": HB_BUFS, "rhb": RHB_BUFS}[tag]
                assert idx > alloc_count[tag] - bufs, \
                    f"bundle {name} slot recycled ({idx} vs {alloc_count[tag]})"
                return t

            def emit_diffusion(src_dram, names, pool_tags):
                """src_dram: node-major bf16 [N, 128*len(names)] DRAM AP."""
                Cm = len(names)
                buns, ps = [], []
                for ti, name in enumerate(names):
                    if name in bundles:
                        buns.append(bundle_get(name))
                    else:
                        pool, tag = pool_tags[ti]
                        buns.append(bundle_alloc(name, pool, tag))
                    p1 = pdiff.tile([128, 512], F32, name=nm("p1"), tag="pdiff")
                    p2 = pdiff.tile([128, 512], F32, name=nm("p2"), tag="pdiff")
                    ps.append((p1, p2))
                KC = 4  # kt tiles per readback chunk
                for ck in range(KT // KC):
                    lt = lhstp.tile([128, KC, Cm * 128], BF16, name=nm("lt"), tag="lt")
                    dma_eng(ck).dma_start(
                        lt,
                        src_dram[ck * KC * 128:(ck + 1) * KC * 128, :]
                        .rearrange("(k p) c -> p k c", p=128),
                    )
                    for k2 in range(KC):
                        kt = ck * KC + k2
                        for ti in range(Cm):
                            p1, p2 = ps[ti]
                            lts = lt[:, k2, ti * 128:(ti + 1) * 128]
                            nc.tensor.matmul(p1, lts, rp1[:, kt, :],
                                             start=(kt == 0), stop=(kt == KT - 1))
                            nc.tensor.matmul(p2, lts, rp2[:, kt, :],
                                             start=(kt == 0), stop=(kt == KT - 1))
                for ti in range(Cm):
                    p1, p2 = ps[ti]
                    bun = buns[ti]
                    for b in range(B):
                        # ops (A, A2) -> [:, b, 1:3, :]; (AT, AT2) -> [:, b, 3:5, :]
                        nc.vector.tensor_copy(bun[:, b, 1:3, :], p1[b * H:(b + 1) * H, :])
                        nc.vector.tensor_copy(bun[:, b, 3:5, :], p2[b * H:(b + 1) * H, :])
                return buns

            def set_identity_slot(bun, src_state):
                nc.vector.tensor_copy(
                    bun[:, :, 0, :],
                    src_state.rearrange("p (b n) -> p b n", b=B),
                )

            def emit_allgather(tensors):
                """tensors: state tiles [64, BN] f32 feature-major. Returns
                gathered node-major bf16 DRAM tile [N, 128*len(tensors)]."""
                Cg = 128 * len(tensors)
                stg = statep.tile([128, 2, Cg], BF16, name=nm("stg"), tag="stg")
                for ti, t in enumerate(tensors):
                    for b in range(B):
                        for nh in range(2):
                            pt = ptr.tile([128, H], F32, name=nm("agt"), tag="ptr")
                            nc.tensor.transpose(
                                pt,
                                t[:, b * NS + nh * 128: b * NS + (nh + 1) * 128],
                                ident[0:H, 0:H],
                            )
                            nc.vector.tensor_copy(
                                stg[:, nh, ti * 128 + b * H: ti * 128 + (b + 1) * H],
                                pt,
                            )
                ag_in = dml.tile([NS, Cg], BF16, name=nm("ag_in"), tag="agin")
                nc.sync.dma_start(ag_in.rearrange("(nh p) c -> p nh c", p=128), stg)
                ag_out = dms.tile([N, Cg], BF16, name=nm("ag_out"), tag="agout",
                                  addr_space="Shared")
                nc.gpsimd.collective_compute(
                    "AllGather", mybir.AluOpType.bypass, replica_groups=RG,
                    ins=[ag_in.opt()], outs=[ag_out.opt()],
                )
                return ag_out

            def proj_mms(wt, bname):
                bun = bundle_get(bname)
                return [(wt[:, op, :], bun[:, :, op, :]) for op in range(5)]

            def emit_mm_chain(psum_out, mms, start, stop):
                pv = psum_out.rearrange("p (b n) -> p b n", b=B)
                for i, (wap, rap) in enumerate(mms):
                    nc.tensor.matmul(pv, wap, rap,
                                     start=(start and i == 0),
                                     stop=(stop and i == len(mms) - 1))

            def emit_projection(psum_out, wx, wh, parts, out_dim):
                mms = []
                for (bname, pos) in parts:
                    if bname is None:
                        continue
                    mms += proj_mms(wx if pos == "x" else wh, bname)
                assert mms
                emit_mm_chain(psum_out, mms, True, True)

            def emit_cell(ph, l, x_name, h_name, cid, extra_gather=(),
                          filler=None):
                key = (ph, l)
                hs_t = state[h_name] if h_name is not None else None
                gps = pproj.tile([2 * H, BN], F32, name=nm("gps"), tag="pproj")
                emit_projection(gps, wgx[key], wgh[key],
                                [(x_name, "x"), (h_name, "h")], 2 * H)
                r_t = statep.tile([H, BN], F32, name=nm("r"), tag="r")
                nc.scalar.activation(r_t, gps[0:H, :], AF.Sigmoid, bias=bgr_sb[key])
                u_t = statep.tile([H, BN], F32, name=nm("u"), tag="u")
                nc.scalar.activation(u_t, gps[H:2 * H, :], AF.Sigmoid, bias=bgu_sb[key])

                cps = pproj.tile([H, BN], F32, name=nm("cps"), tag="pproj")
                if h_name is not None:
                    rh_t = statep.tile([H, BN], F32, name=nm("rh"), tag="rh")
                    nc.vector.tensor_mul(rh_t, r_t, hs_t)
                    rh_name = f"rh_{cid}"
                    # cand x-part first: these matmuls depend only on cached
                    # bundles, so the PE runs them during the rh AllGather.
                    if x_name is not None:
                        emit_mm_chain(cps, proj_mms(wcx[key], x_name), True, False)
                    # extra_gather tensors piggyback on the rh AllGather but
                    # are diffused in a separate chain AFTER the cand-h
                    # matmuls, so they fill PE gaps during the cell tail
                    # instead of delaying the rh bundle.
                    ag = emit_allgather(
                        [rh_t] + [state[nm_] for nm_ in extra_gather])
                    # AG-independent PE work emitted here executes during the
                    # collective, keeping the PE p-state ramped.
                    if filler is not None:
                        filler()
                    buns = emit_diffusion(ag[:, 0:128], [rh_name],
                                          [(rhbp, "rhb")])
                    set_identity_slot(buns[0], rh_t)
                    emit_mm_chain(cps, proj_mms(wch[key], rh_name),
                                  x_name is None, True)
                    if extra_gather:
                        # deferred: the caller emits this AFTER the next
                        # AllGather's doorbell so it fills that mesh window
                        # instead of delaying the doorbell on the PE queue
                        def extras_diff(ag=ag, names=tuple(extra_gather)):
                            ebuns = emit_diffusion(
                                ag[:, 128:], list(names),
                                [(hbp, "hb")] * len(names))
                            for bun, nm_ in zip(ebuns, names):
                                set_identity_slot(bun, state[nm_])
                        pending_extras[0] = extras_diff
                else:
                    rh_name = None
                    emit_projection(cps, wcx[key], wch[key],
                                    [(x_name, "x")], H)
                cand_t = statep.tile([H, BN], F32, name=nm("cand"), tag="cand")
                nc.scalar.activation(cand_t, cps, AF.Tanh, bias=bc_sb[key])

                hn = hstp.tile([H, BN], F32, name=nm("h"), tag="hst")
                tmp = statep.tile([H, BN], F32, name=nm("tmp"), tag="tmp")
                if h_name is None:
                    nc.vector.tensor_mul(tmp, u_t, cand_t)
                    nc.vector.tensor_sub(hn, cand_t, tmp)        # (1-u)*c
                else:
                    tmp2 = statep.tile([H, BN], F32, name=nm("tmp2"), tag="tmp2")
                    nc.vector.tensor_sub(tmp, hs_t, cand_t)
                    nc.vector.tensor_mul(tmp2, u_t, tmp)
                    nc.vector.tensor_add(hn, cand_t, tmp2)       # c + u*(h-c)
                sname = f"h{l}_{cid}"
                state[sname] = hn
                return sname

            def gather_and_diffuse(names, filler=None):
                ag = emit_allgather([state[nm_] for nm_ in names])
                if pending_extras[0] is not None:
                    pending_extras[0]()
                    pending_extras[0] = None
                if filler is not None:
                    filler()
                buns = emit_diffusion(ag[:, :], names, [(hbp, "hb")] * len(names))
                for bun, nm_ in zip(buns, names):
                    set_identity_slot(bun, state[nm_])

            # ---- skew absorber: a tiny AllGather issued before any compute
            # aligns the 8 cores' launch skew here (under the xp prefix work)
            # instead of inside the first real collective on the critical path
            warm_in = dml.tile([1, 128], BF16, name="warm_in", tag="warm")
            warm_out = dms.tile([W, 128], BF16, name="warm_out", tag="warmo",
                                addr_space="Shared")
            nc.gpsimd.collective_compute(
                "AllGather", mybir.AluOpType.bypass, replica_groups=RG,
                ins=[warm_in.opt()], outs=[warm_out.opt()],
            )

            # ---- persistent SBUF ----
            ident = persist.tile([128, 128], F32, name="ident")
            nc.sync.dma_start(ident, ident_in.ap())
            # pre-load the Sigmoid/Tanh activation tables (each first use
            # costs a ~1.5us ACT_TABLE_LOAD stall mid-cell otherwise)
            warm_act = smallp.tile([1, 1], F32, name="warm_act", tag="wact")
            nc.scalar.activation(warm_act, ident[0:1, 0:1], AF.Sigmoid)
            warm_act2 = smallp.tile([1, 1], F32, name="warm_act2", tag="wact2")
            nc.scalar.activation(warm_act2, ident[0:1, 0:1], AF.Tanh)
            # operator pairs: rp1 = [A^T | A^T^2], rp2 = [A | A^2], per kt
            rp1 = persist.tile([128, KT, 512], BF16, name="rp1")
            rp2 = persist.tile([128, KT, 512], BF16, name="rp2")
            nc.sync.dma_start(
                rp1, rpa_in.ap().rearrange("(kt p) n -> p kt n", p=128),
            )
            nc.scalar.dma_start(
                rp2, rpb_in.ap().rearrange("(kt p) n -> p kt n", p=128),
            )
            # weights split into x-part / h-part tiles (base partition 0 each)
            wgx, wgh, wcx, wch, bgr_sb, bgu_sb, bc_sb = {}, {}, {}, {}, {}, {}, {}
            for pi, key in enumerate(PHASES):
                src_g = wg_in.ap()[pi, :, :].rearrange("(o p) u -> p o u", p=128)
                wgx[key] = persist.tile([H, 5, 2 * H], BF16, name=f"wgx{pi}")
                nc.sync.dma_start(wgx[key], src_g[0:H])
                wgh[key] = persist.tile([H, 5, 2 * H], BF16, name=f"wgh{pi}")
                nc.sync.dma_start(wgh[key], src_g[H:2 * H])
                src_c = wc_in.ap()[pi, :, :].rearrange("(o p) u -> p o u", p=128)
                wcx[key] = persist.tile([H, 5, H], BF16, name=f"wcx{pi}")
                nc.sync.dma_start(wcx[key], src_c[0:H])
                wch[key] = persist.tile([H, 5, H], BF16, name=f"wch{pi}")
                nc.sync.dma_start(wch[key], src_c[H:2 * H])
                bgr_sb[key] = persist.tile([H, 1], F32, name=f"bgr{pi}")
                nc.sync.dma_start(bgr_sb[key], bg_in.ap()[pi * 128: pi * 128 + H, :])
                bgu_sb[key] = persist.tile([H, 1], F32, name=f"bgu{pi}")
                nc.sync.dma_start(bgu_sb[key], bg_in.ap()[pi * 128 + H: pi * 128 + 2 * H, :])
                bc_sb[key] = persist.tile([H, 1], F32, name=f"bc{pi}")
                nc.sync.dma_start(bc_sb[key], bc_in.ap()[pi * H: (pi + 1) * H, :])
            wout_sb = persist.tile([H, 1], F32, name="wout_sb")
            nc.sync.dma_start(wout_sb, wout_in.ap())
            bout_sb = persist.tile([1, 1], F32, name="bout_sb")
            nc.sync.dma_start(bout_sb, bout_in.ap())

            # =================================================================
            # XP: identity slots + diffusion of all 3 timesteps (global, no AG)
            # =================================================================
            for t in range(SEQ):
                bun = bundle_alloc(f"xp_t{t}", hbp, "hb")
                nc.sync.dma_start(
                    bun[:, :, 0, :],
                    xp_fm.ap()[:, t * BN:(t + 1) * BN]
                    .rearrange("p (b n) -> p b n", b=B),
                )

            def xp_diff(t):
                return lambda: emit_diffusion(
                    xp_nm.ap()[:, t * 128:(t + 1) * 128], [f"xp_t{t}"], None)

            # xp_t1/t2 diffusions are deferred into AllGather-wait windows so
            # the PE stays busy (and ramped) while collectives are in flight.
            xp_diff(0)()

            # =================================================================
            # cells
            # =================================================================
            emit_cell("enc", 0, "xp_t0", None, "e0l0")
            gather_and_diffuse(["h0_e0l0"], filler=xp_diff(1))
            emit_cell("enc", 1, "h0_e0l0", None, "e0l1")

            h0_prev, h1_prev = "h0_e0l0", "h1_e0l1"
            for t in (1, 2):
                s0 = emit_cell("enc", 0, f"xp_t{t}", h0_prev, f"e{t}l0",
                               extra_gather=[h1_prev],
                               filler=xp_diff(2) if t == 1 else None)
                gather_and_diffuse([s0])
                s1 = emit_cell("enc", 1, s0, h1_prev, f"e{t}l1")
                h0_prev, h1_prev = s0, s1

            d0 = emit_cell("dec", 0, None, h0_prev, "d0l0",
                           extra_gather=[h1_prev])
            gather_and_diffuse([d0])
            d1 = emit_cell("dec", 1, d0, h1_prev, "d0l1")

            # output projection: o = wout.T @ h1_dec + bout -> [1, BN]
            ops = pproj.tile([1, BN], F32, name="ops", tag="pproj")
            nc.tensor.matmul(ops, wout_sb, state[d1], start=True, stop=True)
            out_sb = smallp.tile([1, BN], F32, name="out_sb", tag="outsb")
            nc.vector.tensor_scalar_add(out_sb, ops, bout_sb)
            nc.sync.dma_start(out_t.ap(), out_sb)

    nc.compile()
    return nc


def make_in_maps(inputs):
    adj = np.asarray(inputs["adj"], np.float64)
    A = adj + np.eye(N) * 1e-6
    A = (A / (A.sum(axis=1, keepdims=True) + 1e-8)).astype(np.float32)
    AT = np.ascontiguousarray(A.T)
    A2 = A @ A
    AT2 = np.ascontiguousarray(A2.T)     # (A^T)^2
    xp = (np.asarray(inputs["inputs"], np.float32)[..., None]
          @ np.asarray(inputs["in_proj_w"], np.float32)
          + np.asarray(inputs["in_proj_b"], np.float32))  # (B, SEQ, N, H)
    xp_nm = np.ascontiguousarray(
        xp.transpose(2, 1, 0, 3).reshape(N, SEQ * B * H)).astype(NPBF16)
    wg = np.ascontiguousarray(np.concatenate(
        [np.asarray(inputs["enc_gate_w"], np.float32),
         np.asarray(inputs["dec_gate_w"], np.float32)], axis=0)).astype(NPBF16)
    wc = np.ascontiguousarray(np.concatenate(
        [np.asarray(inputs["enc_cand_w"], np.float32),
         np.asarray(inputs["dec_cand_w"], np.float32)], axis=0)).astype(NPBF16)
    bg = np.ascontiguousarray(np.concatenate(
        [np.asarray(inputs["enc_gate_b"], np.float32),
         np.asarray(inputs["dec_gate_b"], np.float32)], axis=0).reshape(4 * 2 * H, 1))
    bc = np.ascontiguousarray(np.concatenate(
        [np.asarray(inputs["enc_cand_b"], np.float32),
         np.asarray(inputs["dec_cand_b"], np.float32)], axis=0).reshape(4 * H, 1))
    wout = np.ascontiguousarray(np.asarray(inputs["out_proj_w"], np.float32))
    bout = np.asarray(inputs["out_proj_b"], np.float32).reshape(1, 1)
    ident = np.eye(128, dtype=np.float32)

    in_maps = []
    for r in range(W):
        sh = slice(r * NS, (r + 1) * NS)
        xp_fm = np.ascontiguousarray(
            xp[:, :, sh, :].transpose(3, 1, 0, 2).reshape(H, SEQ * B * NS)
        ).astype(NPBF16)
        rpa = np.ascontiguousarray(
            np.concatenate([AT[:, sh], AT2[:, sh]], axis=1)).astype(NPBF16)
        rpb = np.ascontiguousarray(
            np.concatenate([A[:, sh], A2[:, sh]], axis=1)).astype(NPBF16)
        in_maps.append({
            "rpa_in": rpa,
            "rpb_in": rpb,
            "xp_nm": xp_nm,
            "xp_fm": xp_fm,
            "wg_in": wg, "wc_in": wc, "bg_in": bg, "bc_in": bc,
            "wout_in": wout, "bout_in": bout, "ident_in": ident,
        })
    return in_maps


def assemble_output(results):
    out = np.zeros((B, 1, N), np.float32)
    for r in range(W):
        res = results[r]["out"]  # [1, BN]
        for b in range(B):
            out[b, 0, r * NS:(r + 1) * NS] = res[0, b * NS:(b + 1) * NS]
    return out


_CACHE = {}


def get_program():
    if "nc" not in _CACHE:
        _CACHE["nc"] = build_program()
    return _CACHE["nc"]


def kernel(**inputs):
    nc = get_program()
    in_maps = make_in_maps(inputs)
    res = run_bass_kernel_spmd(nc, in_maps, core_ids=list(range(W)))
    return assemble_output(res.results)


# revision 17
# speedup vs baseline: 1.8410x; 1.1615x over previous
"""DCRNN Trainium2 kernel: 8-way node sharding, bf16 PE path, software-
pipelined schedule.

Decomposition:
- A row-normalized on host; A^2 / (A^T)^2 also precomputed on host (they are
  input-only functions of adj, like the normalization itself). Per-core
  operator pair tiles rp1 = [A^T | (A^T)^2][:, sh], rp2 = [A | A^2][:, sh]
  are fed as bf16 inputs — no on-device setup phase.
- Activations live feature-major per shard: state tiles [H=64, (b, n)=512].
- Each diffused tensor gets a "bundle" [64, (b, op5, n256)] bf16 = identity +
  the 4 operator applications, produced by bf16 matmuls whose lhsT is the
  AllGathered node-major activation [2048, 128] (bf16) streamed in 128-row
  chunks against the resident operator pair tiles (rhs [128, 512]).
- Projections contract (op, feat) with K=64 bf16 W slices against bundle
  slices; gates/cand ACT and the GRU update stay f32 row-local DVE/ACT work.

Schedule (the core of the speedup): per timestep the four gathered tensors
(rh_l0, s0, rh_l1, h1) each get their own AllGather whose doorbell rings the
moment the tensor exists. Layer-0 of step t+1 depends only on s0(t) and xp,
so its gates/rh/AllGather run during layer-1 of step t; each collective's
mesh is then covered by the diffusion of the previously gathered tensor.
A tiny warm-up AllGather absorbs the cross-core launch skew before the
first real collective.

Hardware constraints honored (probed on trn2):
- every instruction <= 1 sync wait -> must build on bacc.Bacc + nc.compile()
- 2-input DVE ops and matmul lhsT/rhs need equal base partitions
- DMA cannot read PSUM; transposes bounce PSUM -> DVE copy -> SBUF
- DMA only on sync/scalar/gpsimd queues; gpsimd kept clear for collectives
- PE p-state: 2.4 GHz only after ~3us of gap-free execution (else 1.2)
"""
import numpy as np
import ml_dtypes
import concourse.bass as bass
import concourse.bacc as bacc
import concourse.tile as tile
from concourse import mybir
from concourse.bass_utils import run_bass_kernel_spmd

F32 = mybir.dt.float32
BF16 = mybir.dt.bfloat16
AF = mybir.ActivationFunctionType
NPBF16 = ml_dtypes.bfloat16

N, H, B, SEQ, L = 2048, 64, 2, 3, 2
W = 8            # cores
NS = N // W      # 256 nodes per shard
KT = N // 128    # 16 contraction tiles
BN = B * NS      # 512 = (b, n) free size
RG = [list(range(W))]
PHASES = [("enc", 0), ("enc", 1), ("dec", 0), ("dec", 1)]  # dram row order
HB_BUFS, RHB_BUFS = 4, 2


def build_program():
    nc = bacc.Bacc(None, num_devices=W, name="dcrnn")

    # ---- DRAM inputs (per core) ----
    rpa_in = nc.dram_tensor("rpa_in", [N, 2 * NS], BF16, kind="ExternalInput")
    rpb_in = nc.dram_tensor("rpb_in", [N, 2 * NS], BF16, kind="ExternalInput")
    xp_nm = nc.dram_tensor("xp_nm", [N, SEQ * 128], BF16, kind="ExternalInput")
    xp_fm = nc.dram_tensor("xp_fm", [H, SEQ * BN], BF16, kind="ExternalInput")
    wg_in = nc.dram_tensor("wg_in", [4, 5 * 128, 2 * H], BF16, kind="ExternalInput")
    wc_in = nc.dram_tensor("wc_in", [4, 5 * 128, H], BF16, kind="ExternalInput")
    bg_in = nc.dram_tensor("bg_in", [4 * 2 * H, 1], F32, kind="ExternalInput")
    bc_in = nc.dram_tensor("bc_in", [4 * H, 1], F32, kind="ExternalInput")
    wout_in = nc.dram_tensor("wout_in", [H, 1], F32, kind="ExternalInput")
    bout_in = nc.dram_tensor("bout_in", [1, 1], F32, kind="ExternalInput")
    ident_in = nc.dram_tensor("ident_in", [128, 128], F32, kind="ExternalInput")
    out_t = nc.dram_tensor("out", [1, BN], F32, kind="ExternalOutput")

    with tile.TileContext(nc) as tc:
        with (
            tc.tile_pool(name="persist", bufs=1) as persist,
            tc.tile_pool(name="lhstp", bufs=4) as lhstp,
            tc.tile_pool(name="hbp", bufs=HB_BUFS) as hbp,
            tc.tile_pool(name="rhbp", bufs=RHB_BUFS) as rhbp,
            tc.tile_pool(name="statep", bufs=2) as statep,
            tc.tile_pool(name="hstp", bufs=5) as hstp,
            tc.tile_pool(name="smallp", bufs=2) as smallp,
            tc.tile_pool(name="pdiff", bufs=4, space="PSUM") as pdiff,
            tc.tile_pool(name="pproj", bufs=3, space="PSUM") as pproj,
            tc.tile_pool(name="ptr", bufs=1, space="PSUM") as ptr,
            tc.tile_pool(name="dml", bufs=4, space="DRAM") as dml,
            tc.tile_pool(name="dms", bufs=4, space="DRAM") as dms,
        ):
            uid = [0]

            def nm(pfx):
                uid[0] += 1
                return f"{pfx}{uid[0]}"

            # gpsimd is reserved for collective dispatch: any DMA queued on it
            # would delay the doorbell and stretch the exposed mesh latency
            dma_engines = [nc.sync, nc.scalar]

            def dma_eng(i):
                return dma_engines[i % len(dma_engines)]

            # =================================================================
            # helpers
            # =================================================================
            bundles = {}   # name -> (tile, alloc_idx, tag)
            alloc_count = {"hb": 0, "rhb": 0}
            state = {}     # name -> state tile [64, BN]

            def bundle_alloc(name, pool, tag):
                t = pool.tile([H, B, 5, NS], BF16, name=nm("bun_" + name), tag=tag)
                alloc_count[tag] += 1
                bundles[name] = (t, alloc_count[tag], tag)
                return t

            def bundle_get(name):
                t, idx, tag = bundles[name]
                bufs = {"hb": HB_BUFS, "rhb": RHB_BUFS}[tag]
                assert idx > alloc_count[tag] - bufs, \
                    f"bundle {name} slot recycled ({idx} vs {alloc_count[tag]})"
                return t

            def emit_diffusion(src_dram, names, pool_tags):
                """src_dram: node-major bf16 [N, 128*len(names)] DRAM AP."""
                Cm = len(names)
                buns, ps = [], []
                for ti, name in enumerate(names):
                    if name in bundles:
                        buns.append(bundle_get(name))
                    else:
                        pool, tag = pool_tags[ti]
                        buns.append(bundle_alloc(name, pool, tag))
                    p1 = pdiff.tile([128, 512], F32, name=nm("p1"), tag="pdiff")
                    p2 = pdiff.tile([128, 512], F32, name=nm("p2"), tag="pdiff")
                    ps.append((p1, p2))
                KC = 4  # kt tiles per readback chunk
                for ck in range(KT // KC):
                    lt = lhstp.tile([128, KC, Cm * 128], BF16, name=nm("lt"), tag="lt")
                    dma_eng(ck).dma_start(
                        lt,
                        src_dram[ck * KC * 128:(ck + 1) * KC * 128, :]
                        .rearrange("(k p) c -> p k c", p=128),
                    )
                    for k2 in range(KC):
                        kt = ck * KC + k2
                        for ti in range(Cm):
                            p1, p2 = ps[ti]
                            lts = lt[:, k2, ti * 128:(ti + 1) * 128]
                            nc.tensor.matmul(p1, lts, rp1[:, kt, :],
                                             start=(kt == 0), stop=(kt == KT - 1))
                            nc.tensor.matmul(p2, lts, rp2[:, kt, :],
                                             start=(kt == 0), stop=(kt == KT - 1))
                for ti in range(Cm):
                    p1, p2 = ps[ti]
                    bun = buns[ti]
                    for b in range(B):
                        # ops (A, A2) -> [:, b, 1:3, :]; (AT, AT2) -> [:, b, 3:5, :]
                        nc.vector.tensor_copy(bun[:, b, 1:3, :], p1[b * H:(b + 1) * H, :])
                        nc.vector.tensor_copy(bun[:, b, 3:5, :], p2[b * H:(b + 1) * H, :])
                return buns

            def set_identity_slot(bun, src_state):
                nc.vector.tensor_copy(
                    bun[:, :, 0, :],
                    src_state.rearrange("p (b n) -> p b n", b=B),
                )

            def diffuse(ag, name, src_state, tag="hb"):
                pool = {"hb": hbp, "rhb": rhbp}[tag]
                buns = emit_diffusion(ag[:, :], [name], [(pool, tag)])
                set_identity_slot(buns[0], src_state)

            def emit_allgather(t):
                """t: state tile [64, BN] f32 feature-major. Transposes to
                node-major bf16, DMAs to DRAM, rings the collective doorbell.
                Returns the gathered node-major bf16 DRAM tile [N, 128]."""
                Cg = 128
                # all 4 transposed blocks land in ONE psum bank, one DVE copy
                pt = ptr.tile([128, 2, B, H], F32, name=nm("agt"), tag="ptr")
                for b in range(B):
                    for nh in range(2):
                        nc.tensor.transpose(
                            pt[:, nh, b, :],
                            t[:, b * NS + nh * 128: b * NS + (nh + 1) * 128],
                            ident[0:H, 0:H],
                        )
                stg = statep.tile([128, 2, Cg], BF16, name=nm("stg"), tag="stg")
                nc.vector.tensor_copy(stg, pt)
                ag_in = dml.tile([NS, Cg], BF16, name=nm("ag_in"), tag="agin")
                nc.sync.dma_start(ag_in.rearrange("(nh p) c -> p nh c", p=128), stg)
                ag_out = dms.tile([N, Cg], BF16, name=nm("ag_out"), tag="agout",
                                  addr_space="Shared")
                nc.gpsimd.collective_compute(
                    "AllGather", mybir.AluOpType.bypass, replica_groups=RG,
                    ins=[ag_in.opt()], outs=[ag_out.opt()],
                )
                return ag_out

            def proj_mms(wt, bname):
                bun = bundle_get(bname)
                return [(wt[:, op, :], bun[:, :, op, :]) for op in range(5)]

            def emit_mm_chain(psum_out, mms, start, stop):
                pv = psum_out.rearrange("p (b n) -> p b n", b=B)
                for i, (wap, rap) in enumerate(mms):
                    nc.tensor.matmul(pv, wap, rap,
                                     start=(start and i == 0),
                                     stop=(stop and i == len(mms) - 1))

            def gates_psum():
                return pproj.tile([2 * H, BN], F32, name=nm("gps"), tag="pproj")

            def cand_psum():
                return pproj.tile([H, BN], F32, name=nm("cps"), tag="pproj")

            def sigmoids(key, gps, want_r=True):
                r_t = None
                if want_r:
                    r_t = statep.tile([H, BN], F32, name=nm("r"), tag="r")
                    nc.scalar.activation(r_t, gps[0:H, :], AF.Sigmoid,
                                         bias=bgr_sb[key])
                u_t = statep.tile([H, BN], F32, name=nm("u"), tag="u")
                nc.scalar.activation(u_t, gps[H:2 * H, :], AF.Sigmoid,
                                     bias=bgu_sb[key])
                return r_t, u_t

            def rh_mul(r_t, h_state):
                rh_t = statep.tile([H, BN], F32, name=nm("rh"), tag="rh")
                nc.vector.tensor_mul(rh_t, r_t, h_state)
                return rh_t

            def finish_cell(key, u_t, cps, h_state):
                cand_t = statep.tile([H, BN], F32, name=nm("cand"), tag="cand")
                nc.scalar.activation(cand_t, cps, AF.Tanh, bias=bc_sb[key])
                hn = hstp.tile([H, BN], F32, name=nm("h"), tag="hst")
                tmp = statep.tile([H, BN], F32, name=nm("tmp"), tag="tmp")
                if h_state is None:
                    nc.vector.tensor_mul(tmp, u_t, cand_t)
                    nc.vector.tensor_sub(hn, cand_t, tmp)        # (1-u)*c
                else:
                    tmp2 = statep.tile([H, BN], F32, name=nm("tmp2"), tag="tmp2")
                    nc.vector.tensor_sub(tmp, h_state, cand_t)
                    nc.vector.tensor_mul(tmp2, u_t, tmp)
                    nc.vector.tensor_add(hn, cand_t, tmp2)       # c + u*(h-c)
                return hn

            # ---- skew absorber: a tiny AllGather issued before any compute
            # aligns the 8 cores' launch skew here (under the xp prefix work)
            # instead of inside the first real collective on the critical path
            warm_in = dml.tile([1, 128], BF16, name="warm_in", tag="warm")
            warm_out = dms.tile([W, 128], BF16, name="warm_out", tag="warmo",
                                addr_space="Shared")
            nc.gpsimd.collective_compute(
                "AllGather", mybir.AluOpType.bypass, replica_groups=RG,
                ins=[warm_in.opt()], outs=[warm_out.opt()],
            )

            # ---- persistent SBUF ----
            ident = persist.tile([128, 128], F32, name="ident")
            nc.sync.dma_start(ident, ident_in.ap())
            # pre-load the Sigmoid/Tanh activation tables (each first use
            # costs a ~1.5us ACT_TABLE_LOAD stall mid-cell otherwise)
            warm_act = smallp.tile([1, 1], F32, name="warm_act", tag="wact")
            nc.scalar.activation(warm_act, ident[0:1, 0:1], AF.Sigmoid)
            warm_act2 = smallp.tile([1, 1], F32, name="warm_act2", tag="wact2")
            nc.scalar.activation(warm_act2, ident[0:1, 0:1], AF.Tanh)
            # operator pairs: rp1 = [A^T | A^T^2], rp2 = [A | A^2], per kt
            rp1 = persist.tile([128, KT, 512], BF16, name="rp1")
            rp2 = persist.tile([128, KT, 512], BF16, name="rp2")
            nc.sync.dma_start(
                rp1, rpa_in.ap().rearrange("(kt p) n -> p kt n", p=128),
            )
            nc.scalar.dma_start(
                rp2, rpb_in.ap().rearrange("(kt p) n -> p kt n", p=128),
            )
            # weights split into x-part / h-part tiles (base partition 0 each)
            wgx, wgh, wcx, wch, bgr_sb, bgu_sb, bc_sb = {}, {}, {}, {}, {}, {}, {}
            for pi, key in enumerate(PHASES):
                src_g = wg_in.ap()[pi, :, :].rearrange("(o p) u -> p o u", p=128)
                wgx[key] = persist.tile([H, 5, 2 * H], BF16, name=f"wgx{pi}")
                nc.sync.dma_start(wgx[key], src_g[0:H])
                wgh[key] = persist.tile([H, 5, 2 * H], BF16, name=f"wgh{pi}")
                nc.sync.dma_start(wgh[key], src_g[H:2 * H])
                src_c = wc_in.ap()[pi, :, :].rearrange("(o p) u -> p o u", p=128)
                wcx[key] = persist.tile([H, 5, H], BF16, name=f"wcx{pi}")
                nc.sync.dma_start(wcx[key], src_c[0:H])
                wch[key] = persist.tile([H, 5, H], BF16, name=f"wch{pi}")
                nc.sync.dma_start(wch[key], src_c[H:2 * H])
                bgr_sb[key] = persist.tile([H, 1], F32, name=f"bgr{pi}")
                nc.sync.dma_start(bgr_sb[key], bg_in.ap()[pi * 128: pi * 128 + H, :])
                bgu_sb[key] = persist.tile([H, 1], F32, name=f"bgu{pi}")
                nc.sync.dma_start(bgu_sb[key], bg_in.ap()[pi * 128 + H: pi * 128 + 2 * H, :])
                bc_sb[key] = persist.tile([H, 1], F32, name=f"bc{pi}")
                nc.sync.dma_start(bc_sb[key], bc_in.ap()[pi * H: (pi + 1) * H, :])
            wout_sb = persist.tile([H, 1], F32, name="wout_sb")
            nc.sync.dma_start(wout_sb, wout_in.ap())
            bout_sb = persist.tile([1, 1], F32, name="bout_sb")
            nc.sync.dma_start(bout_sb, bout_in.ap())

            # =================================================================
            # xp bundles: identity slots via DMA; diffusions fill AG meshes
            # =================================================================
            for t in range(SEQ):
                bun = bundle_alloc(f"xp_t{t}", hbp, "hb")
                nc.sync.dma_start(
                    bun[:, :, 0, :],
                    xp_fm.ap()[:, t * BN:(t + 1) * BN]
                    .rearrange("p (b n) -> p b n", b=B),
                )

            def xp_diff(t):
                emit_diffusion(xp_nm.ap()[:, t * 128:(t + 1) * 128],
                               [f"xp_t{t}"], None)

            EK0, EK1 = ("enc", 0), ("enc", 1)
            DK0, DK1 = ("dec", 0), ("dec", 1)

            # =================================================================
            # prologue: t=0 (both layers have zero initial state -> no rh)
            # =================================================================
            xp_diff(0)
            # cell(0,0): x = xp_t0, h = 0
            g00 = gates_psum()
            emit_mm_chain(g00, proj_mms(wgx[EK0], "xp_t0"), True, True)
            _, u00 = sigmoids(EK0, g00, want_r=False)
            c00 = cand_psum()
            emit_mm_chain(c00, proj_mms(wcx[EK0], "xp_t0"), True, True)
            s0_st = finish_cell(EK0, u00, c00, None)           # s0(0)
            ag_s0 = emit_allgather(s0_st)
            xp_diff(1)                                          # fills mesh
            diffuse(ag_s0, "s0_0", s0_st)
            # cell(0,1): x = s0(0) bundle, h = 0
            g01 = gates_psum()
            emit_mm_chain(g01, proj_mms(wgx[EK1], "s0_0"), True, True)
            _, u01 = sigmoids(EK1, g01, want_r=False)
            c01 = cand_psum()
            emit_mm_chain(c01, proj_mms(wcx[EK1], "s0_0"), True, True)
            h1_st = finish_cell(EK1, u01, c01, None)           # h1(0)
            ag_h1 = emit_allgather(h1_st)                      # doorbell now
            # G0(1): gates(enc,0) for t=1  <- {xp_t1, s0_0}; fills h1 mesh
            g10 = gates_psum()
            emit_mm_chain(g10, proj_mms(wgx[EK0], "xp_t1")
                          + proj_mms(wgh[EK0], "s0_0"), True, True)
            r10, u0_t = sigmoids(EK0, g10)
            rh0_st = rh_mul(r10, s0_st)                        # rh0(1)
            ag_rh0 = emit_allgather(rh0_st)
            xp_diff(2)                                          # fills mesh
            cps0 = cand_psum()
            emit_mm_chain(cps0, proj_mms(wcx[EK0], "xp_t1"), True, False)
            cps0_open = True   # x-part emitted, h-part pending

            s0_prev_st, h1_prev_st = s0_st, h1_st
            s0_prev_name, h1_prev_name = "s0_0", "h1_0"

            # =================================================================
            # steady pipeline over t = 1, 2, dec
            # =================================================================
            STEPS = [
                ("1", EK0, EK1, "xp_t1", "xp_t2", EK0),
                ("2", EK0, EK1, "xp_t2", None, DK0),
                ("d", DK0, DK1, None, None, None),
            ]
            for si, (tname, k0, k1, xpn, xpn_next, k0_next) in enumerate(STEPS):
                rh0n, rh1n = f"rh0_{tname}", f"rh1_{tname}"
                s0n, h1n = f"s0_{tname}", f"h1_{tname}"
                # A1: diffuse rh0(t) (its mesh completed during prior fills);
                # A2: diffuse h1(t-1) (mesh runs during A1)
                diffuse(ag_rh0, rh0n, rh0_st, tag="rhb")
                diffuse(ag_h1, h1_prev_name, h1_prev_st)
                # A3: finish cell(t, l0) -> s0(t)
                emit_mm_chain(cps0, proj_mms(wch[k0], rh0n),
                              not cps0_open, True)
                s0_st = finish_cell(k0, u0_t, cps0, s0_prev_st)
                # A4: gather s0(t); A5 fill: gates(t, l1) h-part
                ag_s0 = emit_allgather(s0_st)
                g1 = gates_psum()
                emit_mm_chain(g1, proj_mms(wgh[k1], h1_prev_name), True, False)
                # A6: diffuse s0(t)
                diffuse(ag_s0, s0n, s0_st)
                # B1: gates(t, l1) x-part; rh1(t)
                emit_mm_chain(g1, proj_mms(wgx[k1], s0n), False, True)
                r1, u1 = sigmoids(k1, g1)
                rh1_st = rh_mul(r1, h1_prev_st)
                # B2: gather rh1(t)
                ag_rh1 = emit_allgather(rh1_st)
                # B3 fill: cand(t, l1) x-part + full layer-0 front of t+1
                cps1 = cand_psum()
                emit_mm_chain(cps1, proj_mms(wcx[k1], s0n), True, False)
                if k0_next is not None:
                    g0n = gates_psum()
                    mms = proj_mms(wgh[k0_next], s0n)
                    if xpn_next is not None:
                        mms = proj_mms(wgx[k0_next], xpn_next) + mms
                    emit_mm_chain(g0n, mms, True, True)
                    r0n, u0_t = sigmoids(k0_next, g0n)
                    rh0_st = rh_mul(r0n, s0_st)
                    ag_rh0 = emit_allgather(rh0_st)
                    if xpn_next is not None:
                        cps0 = cand_psum()
                        emit_mm_chain(cps0, proj_mms(wcx[k0_next], xpn_next),
                                      True, False)
                        cps0_open = True
                    else:
                        cps0 = cand_psum()
                        cps0_open = False
                # B5: diffuse rh1(t) (mesh covered by B3)
                diffuse(ag_rh1, rh1n, rh1_st, tag="rhb")
                # B6: finish cell(t, l1) -> h1(t)
                emit_mm_chain(cps1, proj_mms(wch[k1], rh1n), False, True)
                h1_st = finish_cell(k1, u1, cps1, h1_prev_st)
                # B7: gather h1(t) for the next step's gates
                if si < len(STEPS) - 1:
                    ag_h1 = emit_allgather(h1_st)
                s0_prev_st, h1_prev_st = s0_st, h1_st
                s0_prev_name, h1_prev_name = s0n, h1n

            # output projection: o = wout.T @ h1_dec + bout -> [1, BN]
            ops = pproj.tile([1, BN], F32, name="ops", tag="pproj")
            nc.tensor.matmul(ops, wout_sb, h1_st, start=True, stop=True)
            out_sb = smallp.tile([1, BN], F32, name="out_sb", tag="outsb")
            nc.vector.tensor_scalar_add(out_sb, ops, bout_sb)
            nc.sync.dma_start(out_t.ap(), out_sb)

    nc.compile()
    return nc


def make_in_maps(inputs):
    adj = np.asarray(inputs["adj"], np.float64)
    A = adj + np.eye(N) * 1e-6
    A = (A / (A.sum(axis=1, keepdims=True) + 1e-8)).astype(np.float32)
    AT = np.ascontiguousarray(A.T)
    A2 = A @ A
    AT2 = np.ascontiguousarray(A2.T)     # (A^T)^2
    xp = (np.asarray(inputs["inputs"], np.float32)[..., None]
          @ np.asarray(inputs["in_proj_w"], np.float32)
          + np.asarray(inputs["in_proj_b"], np.float32))  # (B, SEQ, N, H)
    xp_nm = np.ascontiguousarray(
        xp.transpose(2, 1, 0, 3).reshape(N, SEQ * B * H)).astype(NPBF16)
    wg = np.ascontiguousarray(np.concatenate(
        [np.asarray(inputs["enc_gate_w"], np.float32),
         np.asarray(inputs["dec_gate_w"], np.float32)], axis=0)).astype(NPBF16)
    wc = np.ascontiguousarray(np.concatenate(
        [np.asarray(inputs["enc_cand_w"], np.float32),
         np.asarray(inputs["dec_cand_w"], np.float32)], axis=0)).astype(NPBF16)
    bg = np.ascontiguousarray(np.concatenate(
        [np.asarray(inputs["enc_gate_b"], np.float32),
         np.asarray(inputs["dec_gate_b"], np.float32)], axis=0).reshape(4 * 2 * H, 1))
    bc = np.ascontiguousarray(np.concatenate(
        [np.asarray(inputs["enc_cand_b"], np.float32),
         np.asarray(inputs["dec_cand_b"], np.float32)], axis=0).reshape(4 * H, 1))
    wout = np.ascontiguousarray(np.asarray(inputs["out_proj_w"], np.float32))
    bout = np.asarray(inputs["out_proj_b"], np.float32).reshape(1, 1)
    ident = np.eye(128, dtype=np.float32)

    in_maps = []
    for r in range(W):
        sh = slice(r * NS, (r + 1) * NS)
        xp_fm = np.ascontiguousarray(
            xp[:, :, sh, :].transpose(3, 1, 0, 2).reshape(H, SEQ * B * NS)
        ).astype(NPBF16)
        rpa = np.ascontiguousarray(
            np.concatenate([AT[:, sh], AT2[:, sh]], axis=1)).astype(NPBF16)
        rpb = np.ascontiguousarray(
            np.concatenate([A[:, sh], A2[:, sh]], axis=1)).astype(NPBF16)
        in_maps.append({
            "rpa_in": rpa,
            "rpb_in": rpb,
            "xp_nm": xp_nm,
            "xp_fm": xp_fm,
            "wg_in": wg, "wc_in": wc, "bg_in": bg, "bc_in": bc,
            "wout_in": wout, "bout_in": bout, "ident_in": ident,
        })
    return in_maps


def assemble_output(results):
    out = np.zeros((B, 1, N), np.float32)
    for r in range(W):
        res = results[r]["out"]  # [1, BN]
        for b in range(B):
            out[b, 0, r * NS:(r + 1) * NS] = res[0, b * NS:(b + 1) * NS]
    return out


_CACHE = {}


def get_program():
    if "nc" not in _CACHE:
        _CACHE["nc"] = build_program()
    return _CACHE["nc"]


def kernel(**inputs):
    nc = get_program()
    in_maps = make_in_maps(inputs)
    res = run_bass_kernel_spmd(nc, in_maps, core_ids=list(range(W)))
    return assemble_output(res.results)
